# revision 40
# baseline (speedup 1.0000x reference)
"""Trainium2 Bass kernel for nn_AttentionModel (4-layer dense transformer).

Contract: kernel(**inputs) takes FULL unsharded inputs (as produced by
setup_inputs) and returns the FULL output [N, L, V] fp32.

Sharding: data-parallel over batch N=8 across the 8 NeuronCores — each core
runs the complete transformer for one batch element (identical NEFF, per-core
tokens). No collectives needed; the host stacks the per-core outputs.

Per-core dataflow (L=1024, F=512, H=8, KD=QD=64, NL=4, V=1024):
  - embedding: indirect-DMA gather of embed rows by token -> x0 natural [L, F]
  - activations kept in two layouts:
      natural [l(128-part) x F]  - for layernorm / residual / softmax scales
      T       [F(128-part) x L]  - as matmul operands (contraction on
      partitions); the T copy consumed by Q/K/V is fp8e4m3 (xT8), the one
      consumed by the unembed stays f32r. PE-transposes convert layouts.
  - per layer (emission interleaved for cross-engine overlap — see the
    schedule comment in the layer loop):
      kT = Wk^T x^T, vT = Wv^T x^T: fp8 DoubleRow matmuls (two 256-deep
           K-passes instead of four 128-deep f32r passes, 4x fewer PE
           cycles); psum pair-tile -> one f32r copy per output chunk
      q  = x Wq fp8 DoubleRow, stored fp16 as [j-chunk, head, 65] with a
           ones column so the attend matmul also produces softmax row-sums
      scores^T[j,i] = v k^T per head in f32r (K=64 matmuls on disjoint PE
           row-groups per head pair; causal tiles only)
      att_u = exp(scores^T - 5) in fp16: both heads of a pair share one
           2-bank psum tile so exp runs as a single wide ACT instruction;
           diagonal tiles triangle-zeroed in place with one gpsimd
           affine_select per pair (keep j<=i)
      x_new[i-block, pair] = att_u^T @ [q | 1] (fp16 matmuls, one psum bank
           per pair): col 64 of each head = softmax row-sum; strided
           reciprocal + 0-stride-broadcast multiply normalize on DVE
      x_newT via PE transposes (4 outputs share one psum bank -> one merged
           copy); MLP h1T = relu(W1^T x_newT + b1) (f32r, ACT relu+bias,
           one [P,2,512] psum pair per output chunk); h = h1T^T W2 (f32r,
           l-block pairs); y = LN(x + h) (bn_stats/bn_aggr on DVE, rstd on
           ACT, apply on gpsimd); yT via PE transposes -> xT8 (fp8) or xTr
           (f32r, last layer, feeds unembed)
  - unembed: logits = x4 Wout + bout in f32r, one [128, 1024] DMA per block.

Engine budget notes: gpsimd (Pool) cannot touch PSUM on TRN2, so all
psum->sbuf traffic is on DVE/ACT (split via TCOPY) and Pool takes the
SBUF-only work (affine_select, LN apply). The ACT table-set choice is pinned
(see _Bacc) so Exp/Ln/Relu/Copy share one loaded set - no per-layer ~2.7us
table swaps. fp8 is limited to the Q/K/V projections: k/v quantization noise
is washed out by the softmax ratio, q noise by attention averaging; scores,
attend, MLP and unembed keep their f32r/fp16 envelopes (measured end-to-end
rel err vs fp32 reference ~3e-3, budget 2e-2).
"""

import numpy as np

import concourse.bass as bass
import concourse.mybir as mybir
import concourse.tile as tile
from concourse import bacc
from concourse.bass_utils import run_bass_kernel_spmd
from concourse.masks import make_identity

# Model dims (hardcoded per the problem spec)
V, F, NL, H, KD, QD = 1024, 512, 4, 8, 64, 64
N, L = 8, 1024
HQ = H * QD  # 512
P = 128
FC = F // P      # 4 f-chunks
LB = L // P      # 8 l-blocks of 128
NCORES = 8

f32 = mybir.dt.float32
f32r = mybir.dt.float32r
f16 = mybir.dt.float16
f8 = mybir.dt.float8e4
i32 = mybir.dt.int32
AF = mybir.ActivationFunctionType
OP = mybir.AluOpType
DR = mybir.MatmulPerfMode.DoubleRow

_NC_CACHE: dict = {}
ABLATE = "none"  # perf-analysis knob: none|scores|attend|transposes
LN_BATCH = False  # batch the LN ln/exp across the 8 l-chunks
TCOPY = "act"  # engine for merged y/x0 transpose copies: dve|act|split.
# Those copies live in the MLP/LN phase where ACT is idle and DVE is the
# phase bottleneck (phase-aware assignment beats global balance).
EXPP_BUFS = 22  # in-flight fp16 att PAIR tiles ([P,2,512]); the interleaved
# schedule keeps pair p's 12 tiles live while pair p+1's 12 are produced
PSUM_CFG = (3, 1, 1)  # bufs for (pp2, pa, pt). pp2 tiles are [P,2,512]
# (2 banks, shared by scores pairs / projection pairs / mlp / unembed); pa
# packs 2 attend accumulators of 130 f32 into one bank; pt packs 4 transpose
# outputs into one bank. Banks: 3*2 + 1 + 1 = 8.


class _Bacc(bacc.Bacc):
    """Bacc with activation-table-set selection pinned to
    natural_log_exp_and_others (contains Exp, Ln, Relu, Copy — everything this
    kernel uses) so the load-insertion pass emits one table load instead of
    thrashing between per-function sets (~2.7us per swap)."""

    def insert_act_table_loads(self):
        from concourse.hw_specs import get_activation_tables
        import concourse.mybir as _mb

        has_activation = any(
            isinstance(i, _mb.InstActivation)
            for b in self.main_func.blocks
            for i in b.instructions
        )
        if not has_activation:
            return
        keep = {AF.Exp, AF.Ln, AF.Relu, AF.Copy}
        chosen = "natural_log_exp_and_others"
        full = get_activation_tables(self.m.arch)
        assert keep <= full[chosen], (chosen, keep - full[chosen])
        tables = [
            (name, (fns if name == chosen else fns - keep))
            for name, fns in full.items()
        ]
        import bass_rust as _bass_rust
        _bass_rust.insert_act_table_loads(self, tables)


def _ln_apply(nc, y, b, mv8, rstd8, use_gamma, use_beta, gamma_b, beta_b):
    t = y[:, b, :]
    # SBUF in-place: runs on gpsimd to keep DVE free for psum traffic
    nc.gpsimd.tensor_scalar(
        t, t, mv8[:, b, 0:1], rstd8[:, b:b + 1],
        op0=OP.subtract, op1=OP.mult)
    if use_gamma:
        nc.gpsimd.tensor_mul(t, t, gamma_b[:])
    if use_beta:
        nc.gpsimd.tensor_add(t, t, beta_b[:])


def _r(ap):
    """View a DRAM fp32 AP as float32r for DMA into f32r tiles."""
    return ap.bitcast(f32r)


def _build(flags, repeat=1):
    use_b1, use_b2, use_gamma, use_beta, use_bout = flags
    nc = _Bacc("TRN2", target_bir_lowering=False, debug=False,
               num_devices=NCORES)

    tokens = nc.declare_dram_parameter("tokens", [L], i32, isOutput=False)
    embed = nc.declare_dram_parameter("embed", [V, F], f32, isOutput=False)
    Wq8 = nc.declare_dram_parameter("Wq8", [NL, F, HQ], f8, isOutput=False)
    Wk8 = nc.declare_dram_parameter("Wk8", [NL, F, H * KD], f8, isOutput=False)
    Wv8 = nc.declare_dram_parameter("Wv8", [NL, F, H * KD], f8, isOutput=False)
    W1 = nc.declare_dram_parameter("W1", [NL, HQ, F], f32, isOutput=False)
    b1 = nc.declare_dram_parameter("b1", [NL, F], f32, isOutput=False)
    W2 = nc.declare_dram_parameter("W2", [NL, F, F], f32, isOutput=False)
    b2 = nc.declare_dram_parameter("b2", [NL, F], f32, isOutput=False)
    gamma = nc.declare_dram_parameter("gamma", [NL, F], f32, isOutput=False)
    beta = nc.declare_dram_parameter("beta", [NL, F], f32, isOutput=False)
    Wout = nc.declare_dram_parameter("Wout", [F, V], f32, isOutput=False)
    bout = nc.declare_dram_parameter("bout", [V], f32, isOutput=False)
    out = nc.declare_dram_parameter("out", [L, V], f32, isOutput=True)

    with tile.TileContext(nc) as tc:
        with (
            tc.tile_pool(name="bigT", bufs=3) as bigT,    # [P, FC, L] f32r
            tc.tile_pool(name="t8", bufs=2) as t8p,       # [P, FC, L] fp8
            tc.tile_pool(name="nat", bufs=3) as natp,     # [P, LB, F] f32
            tc.tile_pool(name="qp", bufs=1) as qp,        # [P, LB, H, 65] f16
            tc.tile_pool(name="expp", bufs=EXPP_BUFS) as expp,  # [P, 2, 512] f16
            tc.tile_pool(name="wp", bufs=4) as wp,
            tc.tile_pool(name="cst", bufs=1) as cst,
            tc.tile_pool(name="sm", bufs=16) as sm,       # small per-partition scalars
            tc.tile_pool(name="op", bufs=2) as outp,      # [P, 1024] out staging
            tc.tile_pool(name="pp2", bufs=PSUM_CFG[0], space="PSUM") as pp2,
            tc.tile_pool(name="pa", bufs=PSUM_CFG[1], space="PSUM") as pa,
            tc.tile_pool(name="pt", bufs=PSUM_CFG[2], space="PSUM") as pt,
        ):
            # ---- constants ----
            ident = cst.tile([P, P], f32, tag="ident")
            make_identity(nc, ident[:])
            eps_t = cst.tile([P, 1], f32, tag="eps")
            nc.vector.memset(eps_t[:], 1e-5)
            neg5_t = cst.tile([P, 1], f32, tag="neg5")
            nc.vector.memset(neg5_t[:], -5.0)
            if use_b1:
                b1_sb = cst.tile([P, NL, FC], f32, tag="b1")
                nc.sync.dma_start(b1_sb[:], b1.rearrange("l (c p) -> p l c", p=P))
            if use_bout:
                bout_b = cst.tile([P, V], f32, tag="bout")
                bout_ap = bout[:]
                nc.sync.dma_start(
                    bout_b[:],
                    bass.AP(tensor=bout_ap.tensor, offset=bout_ap.offset,
                            ap=[[0, P]] + bout_ap.ap),
                )

            def bcast_row(dram_row_ap, tag):
                t = cst.tile([P, F], f32, tag=tag)
                nc.sync.dma_start(
                    t[:],
                    bass.AP(tensor=dram_row_ap.tensor, offset=dram_row_ap.offset,
                            ap=[[0, P]] + dram_row_ap.ap),
                )
                return t

            import contextlib
            _loop = (tc.For_i(0, repeat, 1) if repeat > 1
                     else contextlib.nullcontext())
            with _loop:
                # ---- embedding gather ----
                tok_sb = cst.tile([P, LB], i32, tag="tok")
                nc.sync.dma_start(tok_sb[:], tokens.rearrange("(b p) -> p b", p=P))
                x_nat = natp.tile([P, LB, F], f32, tag="nat")
                for b in range(LB):
                    nc.gpsimd.indirect_dma_start(
                        out=x_nat[:, b, :], out_offset=None,
                        in_=embed[:],
                        in_offset=bass.IndirectOffsetOnAxis(ap=tok_sb[:, b:b + 1], axis=0),
                    )

                def tcopy(i, dst, src):
                    """Merged psum->sbuf copy; engine per TCOPY."""
                    if TCOPY == "dve" or (TCOPY == "split" and i % 2 == 0):
                        nc.vector.tensor_copy(dst, src)
                    else:
                        nc.scalar.copy(dst, src)

                def transpose_blocks(src_nat, dst_T, b0, nb):
                    """Transpose l-blocks [b0, b0+nb) of natural [P, LB, F]
                    f32 into T layout [P, FC, L] (dtype cast per dst tile:
                    f32r or fp8). 4 transpose outputs share one psum bank and
                    drain with a single merged copy."""
                    if ABLATE == "transposes":
                        nc.gpsimd.memset(dst_T[:, :, b0 * P:(b0 + nb) * P], 0.1)
                        return
                    for b in range(b0, b0 + nb):
                        pt_ps = pt.tile([P, FC, P], f32, tag="pt")
                        for c in range(FC):
                            nc.tensor.transpose(
                                pt_ps[:, c, :],
                                src_nat[:, b, c * P:(c + 1) * P], ident[:])
                        tcopy(b, dst_T[:, :, b * P:(b + 1) * P], pt_ps[:])

                xT8 = t8p.tile([P, FC, L], f8, tag="t8")
                transpose_blocks(x_nat, xT8, 0, LB)

                # ---- layers ----
                for li in range(NL):
                    last = li == NL - 1
                    wq8_t = wp.tile([P, FC, HQ], f8, tag="w8", bufs=3)
                    wk8_t = wp.tile([P, FC, HQ], f8, tag="w8", bufs=3)
                    wv8_t = wp.tile([P, FC, HQ], f8, tag="w8", bufs=3)
                    nc.sync.dma_start(wq8_t[:], Wq8[li].rearrange("(c p) o -> p c o", p=P))
                    nc.sync.dma_start(wk8_t[:], Wk8[li].rearrange("(c p) o -> p c o", p=P))
                    nc.sync.dma_start(wv8_t[:], Wv8[li].rearrange("(c p) o -> p c o", p=P))
                    w1_t = wp.tile([P, FC, F], f32r, tag="w", bufs=4)
                    w2_t = wp.tile([P, FC, F], f32r, tag="w", bufs=4)
                    nc.sync.dma_start(w1_t[:], _r(W1[li].rearrange("(c p) o -> p c o", p=P)))
                    nc.sync.dma_start(w2_t[:], _r(W2[li].rearrange("(c p) o -> p c o", p=P)))

                    # kT, vT chunk oc: fp8 DoubleRow (256-deep K per pass),
                    # one [P,2,512] psum pair -> one 1024-wide f32r copy
                    kT = bigT.tile([P, FC, L], f32r, tag="bigT")
                    vT = bigT.tile([P, FC, L], f32r, tag="bigT")

                    def emit_kv(oc):
                        for wi, (w8_t, oT) in enumerate(((wk8_t, kT), (wv8_t, vT))):
                            ps = pp2.tile([P, 2, 512], f32, tag="pp2")
                            for lc in range(2):
                                for fc2 in range(0, FC, 2):
                                    nc.tensor.matmul(
                                        ps[:, lc, :],
                                        w8_t[:, fc2:fc2 + 2, oc * P:(oc + 1) * P],
                                        xT8[:, fc2:fc2 + 2, lc * 512:(lc + 1) * 512],
                                        start=(fc2 == 0), stop=(fc2 == FC - 2),
                                        perf_mode=DR)
                            # chunk 0 drains on ACT: it gates the layer's
                            # first scores while DVE still holds the previous
                            # layer's transpose-copy backlog
                            eng = (nc.scalar.copy if oc == 0
                                   else nc.vector.tensor_copy)
                            eng(oT[:, oc, :],
                                ps[:].rearrange("p a b -> p (a b)"))

                    # q natural (fp16 for the attend matmul), [P(j), jc, head, 65]
                    # with a trailing ones column so attend also yields row-sums
                    q_sb = qp.tile([P, LB, H, 65], f16, tag="q")
                    nc.vector.memset(q_sb[:, :, :, 64:65], 1.0)

                    def emit_q2(bp):
                        ps = pp2.tile([P, 2, 512], f32, tag="pp2")
                        for i2 in range(2):
                            b = 2 * bp + i2
                            for fc2 in range(0, FC, 2):
                                nc.tensor.matmul(
                                    ps[:, i2, :],
                                    xT8[:, fc2:fc2 + 2, b * P:(b + 1) * P],
                                    wq8_t[:, fc2:fc2 + 2, :],
                                    start=(fc2 == 0), stop=(fc2 == FC - 2),
                                    perf_mode=DR)
                        eng = nc.vector.tensor_copy if bp % 2 else nc.scalar.copy
                        eng(q_sb[:, 2 * bp:2 * bp + 2, :, 0:64],
                            ps[:].rearrange("p b (h d) -> p b h d", h=H))

                    x_new = natp.tile([P, LB, F], f32, tag="nat")
                    x_newT = bigT.tile([P, FC, L], f32r, tag="bigT")
                    exp_store: dict = {}

                    def emit_xnewT(p, half):
                        # transposes of x_new chunk p (head pair p's columns)
                        # for the 4 l-blocks finished by attend(p, half).
                        # Pair 3's copies gate MLP1 and run when the exps are
                        # done, so they drain on the then-idle ACT.
                        pt_ps = pt.tile([P, 4, P], f32, tag="pt")
                        for i, b in enumerate(range(4 * half, 4 * half + 4)):
                            nc.tensor.transpose(
                                pt_ps[:, i, :],
                                x_new[:, b, p * P:(p + 1) * P], ident[:])
                        eng = nc.scalar.copy if p == 3 else nc.vector.tensor_copy
                        eng(x_newT[:, p, 4 * half * P:(4 * half + 4) * P],
                            pt_ps[:])

                    def emit_scores(hpair, c):
                        heads = (2 * hpair, 2 * hpair + 1)
                        hc = hpair
                        tiles = {}
                        for jc in range(4 * c + 4):
                            d = jc - 4 * c
                            n0 = 0 if d < 0 else min(P * d, 256)
                            e0 = 0 if d < 0 else P * d
                            # both heads of the pair in one 2-bank psum tile
                            # so exp / affine_select run as single wide
                            # instructions (halves the per-instr ACT access
                            # latency spend)
                            ps = pp2.tile([P, 2, 512], f32, tag="pp2")
                            if ABLATE != "scores":
                                for hi, h in enumerate(heads):
                                    hb = 64 * (h % 2)
                                    nc.tensor.matmul(
                                        ps[:, hi, n0:512],
                                        vT[hb:hb + KD, hc, jc * P:(jc + 1) * P],
                                        kT[hb:hb + KD, hc, c * 512 + n0:(c + 1) * 512],
                                        start=True, stop=True)
                            et = expp.tile([P, 2, 512], f16, tag="exp")
                            # bias=-5: softmax is shift-invariant (both the
                            # attend numerator and the ones-column row-sum
                            # scale by e^-5), keeps exp within fp16 range
                            if ABLATE != "scores":
                                nc.scalar.activation(
                                    et[:, :, e0:512], ps[:, :, e0:512], AF.Exp,
                                    bias=neg5_t[:])
                                if d >= 0:
                                    # zero att where j > i (in-place triangle
                                    # select over both heads at once)
                                    nc.gpsimd.affine_select(
                                        out=et[:, :, e0:e0 + P],
                                        in_=et[:, :, e0:e0 + P],
                                        compare_op=OP.is_ge,
                                        fill=0.0, base=0,
                                        pattern=[[0, 2], [1, P]],
                                        channel_multiplier=-1)
                            else:
                                nc.gpsimd.memset(et[:, :, e0:512], 0.5)
                            tiles[jc] = et
                        exp_store[(hpair, c)] = tiles

                    def emit_attend(hpair, c):
                        heads = (2 * hpair, 2 * hpair + 1)
                        tiles = exp_store.pop((hpair, c))
                        # two b-slots share one psum bank (2 x 130 f32);
                        # normalization is batched per b-pair: one strided
                        # reciprocal + one broadcast multiply for both slots
                        pa_t = pa.tile([P, 2, 130], f32, tag="pa")
                        for b0 in range(4 * c, 4 * c + 4, 2):
                            if ABLATE == "attend":
                                for h in heads:
                                    nc.gpsimd.memset(
                                        x_new[:, b0:b0 + 2,
                                              h * 64:(h + 1) * 64], 0.1)
                                continue
                            for s, b in enumerate((b0, b0 + 1)):
                                lc0 = (b - 4 * c) * P
                                # both heads of the pair accumulate into one
                                # psum bank: head h' at cols [65*h', 65*h'+65)
                                for hi, h in enumerate(heads):
                                    for jc in range(b + 1):
                                        nc.tensor.matmul(
                                            pa_t[:, s, 65 * hi:65 * hi + 65],
                                            tiles[jc][:, hi, lc0:lc0 + P],
                                            q_sb[:, jc, h, :],
                                            start=(jc == 0), stop=(jc == b))
                            pa4 = pa_t[:].rearrange("p s (h x) -> p s h x", h=2)
                            rc = sm.tile([P, 2, 2], f32, tag="rc")
                            nc.vector.reciprocal(rc[:], pa4[:, :, :, 64])
                            # x_new[:, b0:b0+2, pair] = att_u @ q * recip
                            # (recip broadcast 64-wide per head, 0-stride)
                            xdst = x_new[:, b0:b0 + 2,
                                         hpair * P:(hpair + 1) * P].rearrange(
                                "p b (h x) -> p b h x", h=2)
                            nc.vector.tensor_tensor(
                                xdst, pa4[:, :, :, 0:64],
                                rc[:, :, :, None].to_broadcast((P, 2, 2, 64)),
                                OP.mult)

                    # schedule: kv chunk p -> attends of pair p-1 (+ their
                    # transposes) -> scores of pair p. Scores sit last in each
                    # iteration because the pp2 ring throttles them to ACT's
                    # exp pace — everything PE-independent is emitted first.
                    for p in range(H // 2):
                        emit_kv(p)
                        if p == 0:
                            emit_scores(0, 0)
                            emit_scores(0, 1)
                            emit_q2(0)
                            emit_q2(1)
                        else:
                            if p == 1:
                                emit_q2(2)
                                emit_q2(3)
                            emit_attend(p - 1, 0)
                            emit_xnewT(p - 1, 0)
                            emit_attend(p - 1, 1)
                            emit_xnewT(p - 1, 1)
                            emit_scores(p, 0)
                            emit_scores(p, 1)
                    emit_attend(3, 0)
                    emit_xnewT(3, 0)
                    emit_attend(3, 1)
                    emit_xnewT(3, 1)

                    # MLP1: h1T = relu(W1^T x_newT + b1) (f32r), one
                    # [P,2,512] psum pair + one wide DVE relu per out chunk
                    h1T = bigT.tile([P, FC, L], f32r, tag="bigT")
                    for oc in range(FC):
                        ps = pp2.tile([P, 2, 512], f32, tag="pp2")
                        for lc in range(2):
                            for fc in range(FC):
                                nc.tensor.matmul(
                                    ps[:, lc, :],
                                    w1_t[:, fc, oc * P:(oc + 1) * P],
                                    x_newT[:, fc, lc * 512:(lc + 1) * 512],
                                    start=(fc == 0), stop=(fc == FC - 1))
                        bias = b1_sb[:, li, oc:oc + 1] if use_b1 else 0.0
                        nc.scalar.activation(
                            h1T[:, oc, :], ps[:].rearrange("p a b -> p (a b)"),
                            AF.Relu, bias=bias)

                    # MLP2 + residual + LN -> y, l-blocks in pairs, with the
                    # y transposes (and, last layer, the unembed) interleaved
                    # per pair so the layer-boundary chain stays pipelined
                    if use_b2:
                        b2_b = bcast_row(b2[li], f"b2_{li}")
                    if use_gamma:
                        gamma_b = bcast_row(gamma[li], f"g_{li}")
                    if use_beta:
                        beta_b = bcast_row(beta[li], f"be_{li}")
                    if last:
                        wo = []
                        for vc in range(2):
                            wt = wp.tile([P, FC, 512], f32r, tag="w", bufs=4)
                            nc.sync.dma_start(
                                wt[:],
                                _r(Wout[:, vc * 512:(vc + 1) * 512]
                                   .rearrange("(c p) o -> p c o", p=P)))
                            wo.append(wt)
                        xT_next = bigT.tile([P, FC, L], f32r, tag="bigT")
                    else:
                        xT_next = t8p.tile([P, FC, L], f8, tag="t8")
                    y = natp.tile([P, LB, F], f32, tag="nat")
                    mv8 = sm.tile([P, LB, 2], f32, tag="mv8")
                    rstd8 = sm.tile([P, LB], f32, tag="rs8")

                    def emit_unembed(b):
                        ps = pp2.tile([P, 2, 512], f32, tag="pp2")
                        for vc in range(2):
                            for fc in range(FC):
                                nc.tensor.matmul(
                                    ps[:, vc, :],
                                    xT_next[:, fc, b * P:(b + 1) * P],
                                    wo[vc][:, fc, :],
                                    start=(fc == 0), stop=(fc == FC - 1))
                        ot = outp.tile([P, V], f32, tag="o")
                        psf = ps[:].rearrange("p a b -> p (a b)")
                        if use_bout:
                            nc.vector.tensor_add(ot[:], psf, bout_b[:])
                        else:
                            eng = (nc.vector.tensor_copy if b % 2
                                   else nc.scalar.copy)
                            eng(ot[:], psf)
                        nc.sync.dma_start(out[b * P:(b + 1) * P, :], ot[:])

                    for bp in range(LB // 2):
                        ps = pp2.tile([P, 2, 512], f32, tag="pp2")
                        for i2 in range(2):
                            b = 2 * bp + i2
                            for fc in range(FC):
                                nc.tensor.matmul(
                                    ps[:, i2, :],
                                    h1T[:, fc, b * P:(b + 1) * P],
                                    w2_t[:, fc, :],
                                    start=(fc == 0), stop=(fc == FC - 1))
                        t2 = y[:, 2 * bp:2 * bp + 2, :]
                        nc.vector.tensor_add(t2, ps[:], x_nat[:, 2 * bp:2 * bp + 2, :])
                        if use_b2:
                            nc.vector.tensor_add(
                                t2, t2,
                                b2_b[:, None, :].to_broadcast((P, 2, F)))
                        for i2 in range(2):
                            b = 2 * bp + i2
                            st = sm.tile([P, 6], f32, tag="st")
                            nc.vector.bn_stats(st[:], y[:, b, :])
                            nc.vector.bn_aggr(mv8[:, b, :], st[:])
                            # rstd = exp(-0.5*ln(var+eps)) per chunk: the
                            # layer tail stays pipelined
                            nc.scalar.activation(
                                rstd8[:, b:b + 1], mv8[:, b, 1:2], AF.Ln,
                                bias=eps_t[:])
                            nc.scalar.activation(
                                rstd8[:, b:b + 1], rstd8[:, b:b + 1], AF.Exp,
                                scale=-0.5)
                            _ln_apply(nc, y, b, mv8, rstd8, use_gamma,
                                      use_beta,
                                      gamma_b if use_gamma else None,
                                      beta_b if use_beta else None)
                        transpose_blocks(y, xT_next, 2 * bp, 2)
                        if last:
                            emit_unembed(2 * bp)
                            emit_unembed(2 * bp + 1)

                    x_nat = y
                    xT8 = xT_next
    nc.compile()
    return nc


def _get_nc(flags, repeat=1):
    key = (flags, repeat, ABLATE, LN_BATCH, PSUM_CFG, TCOPY, EXPP_BUFS)
    if key not in _NC_CACHE:
        _NC_CACHE[key] = _build(flags, repeat)
    return _NC_CACHE[key]


def make_runner(flags, in_maps, repeat=1):
    """Build a reusable jitted SPMD runner with device-resident inputs.

    Returns (run, split_outputs) where run() executes the kernel once on all
    8 cores and blocks; used by test.py for timing without per-call host->device
    input transfer.
    """
    import jax
    from jax.sharding import Mesh, PartitionSpec, NamedSharding
    from concourse import bass2jax, mybir as _mybir

    bass2jax.install_neuronx_cc_hook()
    nc = _get_nc(flags, repeat)
    partition_name = (nc.partition_id_tensor.name if nc.partition_id_tensor
                      else None)
    in_names, out_names, out_avals, zero_outs = [], [], [], []
    for alloc in nc.m.functions[0].allocations:
        if not isinstance(alloc, _mybir.MemoryLocationSet):
            continue
        name = alloc.memorylocations[0].name
        if alloc.kind == "ExternalInput":
            if name != partition_name:
                in_names.append(name)
        elif alloc.kind == "ExternalOutput":
            shape = tuple(alloc.tensor_shape)
            dtype = _mybir.dt.np(alloc.dtype)
            out_names.append(name)
            out_avals.append(jax.core.ShapedArray(shape, dtype))
            zero_outs.append(np.zeros(shape, dtype))
    n_params = len(in_names)
    n_outs = len(out_avals)
    all_names = in_names + out_names + ([partition_name] if partition_name else [])

    def _body(*args):
        operands = list(args)
        if partition_name is not None:
            operands.append(bass2jax.partition_id_tensor())
        outs = bass2jax._bass_exec_p.bind(
            *operands,
            out_avals=tuple(out_avals),
            in_names=tuple(all_names),
            out_names=tuple(out_names),
            lowering_input_output_aliases=(),
            sim_require_finite=True,
            sim_require_nnan=True,
            nc=nc,
        )
        return tuple(outs)

    from jax.experimental.shard_map import shard_map
    devices = jax.devices()[:NCORES]
    mesh = Mesh(np.asarray(devices), ("core",))
    in_specs = (PartitionSpec("core"),) * (n_params + n_outs)
    out_specs = (PartitionSpec("core"),) * n_outs
    sharded = jax.jit(
        shard_map(_body, mesh=mesh, in_specs=in_specs, out_specs=out_specs,
                  check_rep=False),
        keep_unused=True,
    )
    concat_in = [
        np.concatenate([np.asarray(in_maps[c][nm])[None] for c in range(NCORES)],
                       axis=0).reshape(NCORES * np.asarray(in_maps[0][nm]).shape[0],
                                       *np.asarray(in_maps[0][nm]).shape[1:])
        for nm in in_names
    ]
    sh = NamedSharding(mesh, PartitionSpec("core"))
    dev_in = [jax.device_put(x, sh) for x in concat_in]
    dev_zeros = [
        jax.device_put(np.zeros((NCORES * z.shape[0], *z.shape[1:]), z.dtype), sh)
        for z in zero_outs
    ]

    def run():
        outs = sharded(*dev_in, *dev_zeros)
        jax.block_until_ready(outs)
        return outs

    def split(outs):
        return [
            {nm: np.asarray(outs[i]).reshape(NCORES, *out_avals[i].shape)[c]
             for i, nm in enumerate(out_names)}
            for c in range(NCORES)
        ]

    return run, split


def prep_args(inputs):
    """Host-side arg prep shared by kernel() and test.py: fp32 copies of the
    fp32 params, fp8e4m3 casts of Wq/Wk/Wv (consumed by the DoubleRow
    projection matmuls)."""
    import ml_dtypes
    args = {k: np.ascontiguousarray(np.asarray(v), dtype=np.float32)
            for k, v in inputs.items() if k not in ("tokens", "Wq", "Wk", "Wv")}
    for k in ("Wq", "Wk", "Wv"):
        args[k + "8"] = np.ascontiguousarray(
            np.asarray(inputs[k], dtype=np.float32).astype(ml_dtypes.float8_e4m3))
    return args


def kernel(**inputs) -> np.ndarray:
    tokens = np.asarray(inputs["tokens"])
    args = prep_args(inputs)
    flags = (
        bool(np.any(args["b1"])),
        bool(np.any(args["b2"])),
        bool(np.any(args["gamma"] != 1.0)),
        bool(np.any(args["beta"])),
        bool(np.any(args["bout"])),
    )
    nc = _get_nc(flags)
    tok32 = np.ascontiguousarray(tokens.astype(np.int32))
    in_maps = [dict(args, tokens=tok32[c]) for c in range(NCORES)]
    res = run_bass_kernel_spmd(nc, in_maps, list(range(NCORES)))
    return np.stack([res.results[c]["out"] for c in range(NCORES)], axis=0)


if __name__ == "__main__":
    rng = np.random.default_rng(0)
    toy = {
        "tokens": rng.integers(0, V, size=(N, L)),
        "embed": rng.standard_normal((V, F)).astype(np.float32) * 0.02,
        "Wq": rng.standard_normal((NL, F, HQ)).astype(np.float32) * 0.02,
        "Wk": rng.standard_normal((NL, F, H * KD)).astype(np.float32) * 0.02,
        "Wv": rng.standard_normal((NL, F, H * KD)).astype(np.float32) * 0.02,
        "W1": rng.standard_normal((NL, HQ, F)).astype(np.float32) * 0.02,
        "b1": np.zeros((NL, F), np.float32),
        "W2": rng.standard_normal((NL, F, F)).astype(np.float32) * 0.02,
        "b2": np.zeros((NL, F), np.float32),
        "gamma": np.ones((NL, F), np.float32),
        "beta": np.zeros((NL, F), np.float32),
        "Wout": rng.standard_normal((F, V)).astype(np.float32) * 0.02,
        "bout": np.zeros((V,), np.float32),
    }
    o = kernel(**toy)
    print("out:", o.shape, o.dtype, float(np.abs(o).max()))


# revision 53
# speedup vs baseline: 1.1995x; 1.1995x over previous
"""Trainium2 Bass kernel for nn_AttentionModel (4-layer dense transformer).

Contract: kernel(**inputs) takes FULL unsharded inputs (as produced by
setup_inputs) and returns the FULL output [N, L, V] fp32.

Sharding: data-parallel over batch N=8 across the 8 NeuronCores — each core
runs the complete transformer for one batch element (identical NEFF, per-core
tokens). No collectives needed; the host stacks the per-core outputs.

Per-core dataflow (L=1024, F=512, H=8, KD=QD=64, NL=4, V=1024):
  - embedding: indirect-DMA gather of embed rows by token -> x0 natural [L, F]
  - activations kept in two layouts:
      natural [l(128-part) x F]  - for layernorm / residual / softmax scales
      T       [F(128-part) x L]  - as matmul operands (contraction on
      partitions); the T copy consumed by Q/K/V is fp8e4m3 (xT8), the one
      consumed by the unembed stays f32r. PE-transposes convert layouts.
  - per layer (emission interleaved for cross-engine overlap — see the
    schedule comment in the layer loop):
      kT = Wk^T x^T, vT = Wv^T x^T: fp8 DoubleRow matmuls (two 256-deep
           K-passes instead of four 128-deep f32r passes, 4x fewer PE
           cycles); psum pair-tile -> one f32r copy per output chunk
      q  = x Wq fp8 DoubleRow, stored fp16 as [j-chunk, head, 65] with a
           ones column so the attend matmul also produces softmax row-sums
      scores^T[j,i] = v k^T per head in f32r (K=64 matmuls on disjoint PE
           row-groups per head pair; causal tiles only)
      att_u = exp(scores^T - 5) in fp16: both heads of a pair share one
           2-bank psum tile so exp runs as a single wide ACT instruction;
           diagonal tiles triangle-zeroed in place with one gpsimd
           affine_select per pair (keep j<=i)
      x_new[i-block, pair] = att_u^T @ [q | 1] (fp16 matmuls, one psum bank
           per pair): col 64 of each head = softmax row-sum; strided
           reciprocal + 0-stride-broadcast multiply normalize on DVE
      x_newT via PE transposes (4 outputs share one psum bank -> one merged
           copy); MLP h1T = relu(W1^T x_newT + b1) (f32r, ACT relu+bias,
           one [P,2,512] psum pair per output chunk); h = h1T^T W2 (f32r,
           l-block pairs); y = LN(x + h) (bn_stats/bn_aggr on DVE, rstd on
           ACT, apply on gpsimd); yT via PE transposes -> xT8 (fp8) or xTr
           (f32r, last layer, feeds unembed)
  - unembed: logits = x4 Wout + bout in f32r, one [128, 1024] DMA per block.

Engine budget notes: gpsimd (Pool) cannot touch PSUM on TRN2, so all
psum->sbuf traffic is on DVE/ACT (split via TCOPY) and Pool takes the
SBUF-only work (affine_select, LN apply). The ACT table-set choice is pinned
(see _Bacc) so Exp/Ln/Relu/Copy share one loaded set - no per-layer ~2.7us
table swaps. fp8 is limited to the Q/K/V projections: k/v quantization noise
is washed out by the softmax ratio, q noise by attention averaging; scores,
attend, MLP and unembed keep their f32r/fp16 envelopes (measured end-to-end
rel err vs fp32 reference ~3e-3, budget 2e-2).
"""

import numpy as np

import concourse.bass as bass
import concourse.mybir as mybir
import concourse.tile as tile
from concourse import bacc
from concourse.bass_utils import run_bass_kernel_spmd
from concourse.masks import make_identity

# Model dims (hardcoded per the problem spec)
V, F, NL, H, KD, QD = 1024, 512, 4, 8, 64, 64
N, L = 8, 1024
HQ = H * QD  # 512
P = 128
FC = F // P      # 4 f-chunks
LB = L // P      # 8 l-blocks of 128
NCORES = 8

f32 = mybir.dt.float32
f32r = mybir.dt.float32r
f16 = mybir.dt.float16
f8 = mybir.dt.float8e4
i32 = mybir.dt.int32
AF = mybir.ActivationFunctionType
OP = mybir.AluOpType
DR = mybir.MatmulPerfMode.DoubleRow

_NC_CACHE: dict = {}
ABLATE = "none"  # perf-analysis knob: none|scores|attend|transposes
DR_MODE = "dr"  # q/k/v projection matmul mode: dr (fp8 DoubleRow, 256-deep
# K per pass) | fp8 (plain fp8, 128-deep chunks — isolates DoubleRow's real
# hw throughput from the fp8 layout changes)
MLP_FP8 = True  # MLP1/MLP2 in fp8 DoubleRow (x_newT/h1T stored fp8);
# False keeps the f32r MLP path
LN_BATCH = False  # batch the LN ln/exp across the 8 l-chunks
TCOPY = "act"  # engine for merged y/x0 transpose copies: dve|act|split.
# Those copies live in the MLP/LN phase where ACT is idle and DVE is the
# phase bottleneck (phase-aware assignment beats global balance).
EXPP_BUFS = 22  # in-flight fp16 att PAIR tiles ([P,2,512]); the interleaved
# schedule keeps pair p's 12 tiles live while pair p+1's 12 are produced
PSUM_CFG = (3, 1, 1)  # bufs for (pp2, pa, pt). pp2 tiles are [P,2,512]
# (2 banks, shared by scores pairs / projection pairs / mlp / unembed); pa
# packs 2 attend accumulators of 130 f32 into one bank; pt packs 4 transpose
# outputs into one bank. Banks: 3*2 + 1 + 1 = 8.


class _Bacc(bacc.Bacc):
    """Bacc with activation-table-set selection pinned to
    natural_log_exp_and_others (contains Exp, Ln, Relu, Copy — everything this
    kernel uses) so the load-insertion pass emits one table load instead of
    thrashing between per-function sets (~2.7us per swap)."""

    def insert_act_table_loads(self):
        from concourse.hw_specs import get_activation_tables
        import concourse.mybir as _mb

        has_activation = any(
            isinstance(i, _mb.InstActivation)
            for b in self.main_func.blocks
            for i in b.instructions
        )
        if not has_activation:
            return
        keep = {AF.Exp, AF.Ln, AF.Relu, AF.Copy}
        chosen = "natural_log_exp_and_others"
        full = get_activation_tables(self.m.arch)
        assert keep <= full[chosen], (chosen, keep - full[chosen])
        tables = [
            (name, (fns if name == chosen else fns - keep))
            for name, fns in full.items()
        ]
        import bass_rust as _bass_rust
        _bass_rust.insert_act_table_loads(self, tables)


def _ln_apply(nc, y, b, mv8, rstd8, use_gamma, use_beta, gamma_b, beta_b):
    t = y[:, b, :]
    # SBUF in-place: runs on gpsimd to keep DVE free for psum traffic
    nc.gpsimd.tensor_scalar(
        t, t, mv8[:, b, 0:1], rstd8[:, b:b + 1],
        op0=OP.subtract, op1=OP.mult)
    if use_gamma:
        nc.gpsimd.tensor_mul(t, t, gamma_b[:])
    if use_beta:
        nc.gpsimd.tensor_add(t, t, beta_b[:])


def _r(ap):
    """View a DRAM fp32 AP as float32r for DMA into f32r tiles."""
    return ap.bitcast(f32r)


def _build(flags, repeat=1):
    use_b1, use_b2, use_gamma, use_beta, use_bout = flags
    nc = _Bacc("TRN2", target_bir_lowering=False, debug=False,
               num_devices=NCORES)

    tokens = nc.declare_dram_parameter("tokens", [L], i32, isOutput=False)
    embed = nc.declare_dram_parameter("embed", [V, F], f32, isOutput=False)
    Wq8 = nc.declare_dram_parameter("Wq8", [NL, F, HQ], f8, isOutput=False)
    Wk8 = nc.declare_dram_parameter("Wk8", [NL, F, H * KD], f8, isOutput=False)
    Wv8 = nc.declare_dram_parameter("Wv8", [NL, F, H * KD], f8, isOutput=False)
    if MLP_FP8:
        W18 = nc.declare_dram_parameter("W18", [NL, HQ, F], f8, isOutput=False)
        W28 = nc.declare_dram_parameter("W28", [NL, F, F], f8, isOutput=False)
    W1 = nc.declare_dram_parameter("W1", [NL, HQ, F], f32, isOutput=False)
    b1 = nc.declare_dram_parameter("b1", [NL, F], f32, isOutput=False)
    W2 = nc.declare_dram_parameter("W2", [NL, F, F], f32, isOutput=False)
    b2 = nc.declare_dram_parameter("b2", [NL, F], f32, isOutput=False)
    gamma = nc.declare_dram_parameter("gamma", [NL, F], f32, isOutput=False)
    beta = nc.declare_dram_parameter("beta", [NL, F], f32, isOutput=False)
    Wout = nc.declare_dram_parameter("Wout", [F, V], f32, isOutput=False)
    bout = nc.declare_dram_parameter("bout", [V], f32, isOutput=False)
    out = nc.declare_dram_parameter("out", [L, V], f32, isOutput=True)

    with tile.TileContext(nc) as tc:
        with (
            tc.tile_pool(name="bigT", bufs=3) as bigT,    # [P, FC, L] f32r
            tc.tile_pool(name="t8", bufs=4) as t8p,       # [P, FC, L] fp8
            tc.tile_pool(name="nat", bufs=3) as natp,     # [P, LB, F] f32
            tc.tile_pool(name="qp", bufs=1) as qp,        # [P, LB, H, 65] f16
            tc.tile_pool(name="expp", bufs=EXPP_BUFS) as expp,  # [P, 2, 512] f16
            tc.tile_pool(name="wp", bufs=4) as wp,
            tc.tile_pool(name="cst", bufs=1) as cst,
            tc.tile_pool(name="sm", bufs=16) as sm,       # small per-partition scalars
            tc.tile_pool(name="op", bufs=2) as outp,      # [P, 1024] out staging
            tc.tile_pool(name="pp2", bufs=PSUM_CFG[0], space="PSUM") as pp2,
            tc.tile_pool(name="pa", bufs=PSUM_CFG[1], space="PSUM") as pa,
            tc.tile_pool(name="pt", bufs=PSUM_CFG[2], space="PSUM") as pt,
        ):
            # ---- constants ----
            ident = cst.tile([P, P], f32, tag="ident")
            make_identity(nc, ident[:])
            eps_t = cst.tile([P, 1], f32, tag="eps")
            nc.vector.memset(eps_t[:], 1e-5)
            neg5_t = cst.tile([P, 1], f32, tag="neg5")
            nc.vector.memset(neg5_t[:], -5.0)
            if use_b1:
                b1_sb = cst.tile([P, NL, FC], f32, tag="b1")
                nc.sync.dma_start(b1_sb[:], b1.rearrange("l (c p) -> p l c", p=P))
            if use_bout:
                bout_b = cst.tile([P, V], f32, tag="bout")
                bout_ap = bout[:]
                nc.sync.dma_start(
                    bout_b[:],
                    bass.AP(tensor=bout_ap.tensor, offset=bout_ap.offset,
                            ap=[[0, P]] + bout_ap.ap),
                )

            def bcast_row(dram_row_ap, tag):
                t = cst.tile([P, F], f32, tag=tag)
                nc.sync.dma_start(
                    t[:],
                    bass.AP(tensor=dram_row_ap.tensor, offset=dram_row_ap.offset,
                            ap=[[0, P]] + dram_row_ap.ap),
                )
                return t

            import contextlib
            _loop = (tc.For_i(0, repeat, 1) if repeat > 1
                     else contextlib.nullcontext())
            with _loop:
                # ---- embedding gather ----
                tok_sb = cst.tile([P, LB], i32, tag="tok")
                nc.sync.dma_start(tok_sb[:], tokens.rearrange("(b p) -> p b", p=P))
                x_nat = natp.tile([P, LB, F], f32, tag="nat")
                for b in range(LB):
                    nc.gpsimd.indirect_dma_start(
                        out=x_nat[:, b, :], out_offset=None,
                        in_=embed[:],
                        in_offset=bass.IndirectOffsetOnAxis(ap=tok_sb[:, b:b + 1], axis=0),
                    )

                def tcopy(i, dst, src):
                    """Merged psum->sbuf copy; engine per TCOPY."""
                    if TCOPY == "dve" or (TCOPY == "split" and i % 2 == 0):
                        nc.vector.tensor_copy(dst, src)
                    else:
                        nc.scalar.copy(dst, src)

                def transpose_blocks(src_nat, dst_T, b0, nb):
                    """Transpose l-blocks [b0, b0+nb) of natural [P, LB, F]
                    f32 into T layout [P, FC, L] (dtype cast per dst tile:
                    f32r or fp8). 4 transpose outputs share one psum bank and
                    drain with a single merged copy."""
                    if ABLATE == "transposes":
                        nc.gpsimd.memset(dst_T[:, :, b0 * P:(b0 + nb) * P], 0.1)
                        return
                    for b in range(b0, b0 + nb):
                        pt_ps = pt.tile([P, FC, P], f32, tag="pt")
                        for c in range(FC):
                            nc.tensor.transpose(
                                pt_ps[:, c, :],
                                src_nat[:, b, c * P:(c + 1) * P], ident[:])
                        tcopy(b, dst_T[:, :, b * P:(b + 1) * P], pt_ps[:])

                xT8 = t8p.tile([P, FC, L], f8, tag="t8")
                transpose_blocks(x_nat, xT8, 0, LB)

                # ---- layers ----
                for li in range(NL):
                    last = li == NL - 1
                    wq8_t = wp.tile([P, FC, HQ], f8, tag="w8", bufs=5)
                    wk8_t = wp.tile([P, FC, HQ], f8, tag="w8", bufs=5)
                    wv8_t = wp.tile([P, FC, HQ], f8, tag="w8", bufs=5)
                    nc.sync.dma_start(wq8_t[:], Wq8[li].rearrange("(c p) o -> p c o", p=P))
                    nc.sync.dma_start(wk8_t[:], Wk8[li].rearrange("(c p) o -> p c o", p=P))
                    nc.sync.dma_start(wv8_t[:], Wv8[li].rearrange("(c p) o -> p c o", p=P))
                    if MLP_FP8:
                        w1_t = wp.tile([P, FC, F], f8, tag="w8", bufs=5)
                        w2_t = wp.tile([P, FC, F], f8, tag="w8", bufs=5)
                        nc.sync.dma_start(
                            w1_t[:], W18[li].rearrange("(c p) o -> p c o", p=P))
                        nc.sync.dma_start(
                            w2_t[:], W28[li].rearrange("(c p) o -> p c o", p=P))
                    else:
                        w1_t = wp.tile([P, FC, F], f32r, tag="w", bufs=4)
                        w2_t = wp.tile([P, FC, F], f32r, tag="w", bufs=4)
                        nc.sync.dma_start(
                            w1_t[:], _r(W1[li].rearrange("(c p) o -> p c o", p=P)))
                        nc.sync.dma_start(
                            w2_t[:], _r(W2[li].rearrange("(c p) o -> p c o", p=P)))

                    # kT, vT chunk oc: fp8 DoubleRow (256-deep K per pass),
                    # one [P,2,512] psum pair -> one 1024-wide f32r copy
                    kT = bigT.tile([P, FC, L], f32r, tag="bigT")
                    vT = bigT.tile([P, FC, L], f32r, tag="bigT")

                    def emit_kv(oc):
                        for wi, (w8_t, oT) in enumerate(((wk8_t, kT), (wv8_t, vT))):
                            ps = pp2.tile([P, 2, 512], f32, tag="pp2")
                            if DR_MODE == "dr":
                                # fc2 outer / lc inner: consecutive matmuls
                                # share the stationary -> one ldweights per
                                # weight block
                                for fc2 in range(0, FC, 2):
                                    for lc in range(2):
                                        nc.tensor.matmul(
                                            ps[:, lc, :],
                                            w8_t[:, fc2:fc2 + 2, oc * P:(oc + 1) * P],
                                            xT8[:, fc2:fc2 + 2, lc * 512:(lc + 1) * 512],
                                            start=(fc2 == 0), stop=(fc2 == FC - 2),
                                            perf_mode=DR)
                            else:
                                for fc in range(FC):
                                    for lc in range(2):
                                        nc.tensor.matmul(
                                            ps[:, lc, :],
                                            w8_t[:, fc, oc * P:(oc + 1) * P],
                                            xT8[:, fc, lc * 512:(lc + 1) * 512],
                                            start=(fc == 0), stop=(fc == FC - 1))
                            # chunk 0 drains on ACT: it gates the layer's
                            # first scores while DVE still holds the previous
                            # layer's transpose-copy backlog
                            eng = (nc.scalar.copy if oc == 0
                                   else nc.vector.tensor_copy)
                            eng(oT[:, oc, :],
                                ps[:].rearrange("p a b -> p (a b)"))

                    # q natural (fp16 for the attend matmul), [P(j), jc, head, 65]
                    # with a trailing ones column so attend also yields row-sums
                    q_sb = qp.tile([P, LB, H, 65], f16, tag="q")
                    nc.vector.memset(q_sb[:, :, :, 64:65], 1.0)

                    def emit_q2(bp):
                        ps = pp2.tile([P, 2, 512], f32, tag="pp2")
                        for i2 in range(2):
                            b = 2 * bp + i2
                            if DR_MODE == "dr":
                                for fc2 in range(0, FC, 2):
                                    nc.tensor.matmul(
                                        ps[:, i2, :],
                                        xT8[:, fc2:fc2 + 2, b * P:(b + 1) * P],
                                        wq8_t[:, fc2:fc2 + 2, :],
                                        start=(fc2 == 0), stop=(fc2 == FC - 2),
                                        perf_mode=DR)
                            else:
                                for fc in range(FC):
                                    nc.tensor.matmul(
                                        ps[:, i2, :],
                                        xT8[:, fc, b * P:(b + 1) * P],
                                        wq8_t[:, fc, :],
                                        start=(fc == 0), stop=(fc == FC - 1))
                        eng = nc.vector.tensor_copy if bp % 2 else nc.scalar.copy
                        eng(q_sb[:, 2 * bp:2 * bp + 2, :, 0:64],
                            ps[:].rearrange("p b (h d) -> p b h d", h=H))

                    x_new = natp.tile([P, LB, F], f32, tag="nat")
                    if MLP_FP8:
                        x_newT = t8p.tile([P, FC, L], f8, tag="t8")
                    else:
                        x_newT = bigT.tile([P, FC, L], f32r, tag="bigT")
                    exp_store: dict = {}

                    def emit_xnewT(p, half):
                        # transposes of x_new chunk p (head pair p's columns)
                        # for the 4 l-blocks finished by attend(p, half).
                        # Pair 3's copies gate MLP1 and run when the exps are
                        # done, so they drain on the then-idle ACT.
                        pt_ps = pt.tile([P, 4, P], f32, tag="pt")
                        for i, b in enumerate(range(4 * half, 4 * half + 4)):
                            nc.tensor.transpose(
                                pt_ps[:, i, :],
                                x_new[:, b, p * P:(p + 1) * P], ident[:])
                        eng = nc.scalar.copy if p == 3 else nc.vector.tensor_copy
                        eng(x_newT[:, p, 4 * half * P:(4 * half + 4) * P],
                            pt_ps[:])

                    def emit_scores(hpair, c):
                        heads = (2 * hpair, 2 * hpair + 1)
                        hc = hpair
                        tiles = {}
                        for jc in range(4 * c + 4):
                            d = jc - 4 * c
                            n0 = 0 if d < 0 else min(P * d, 256)
                            e0 = 0 if d < 0 else P * d
                            # both heads of the pair in one 2-bank psum tile
                            # so exp / affine_select run as single wide
                            # instructions (halves the per-instr ACT access
                            # latency spend)
                            ps = pp2.tile([P, 2, 512], f32, tag="pp2")
                            if ABLATE != "scores":
                                for hi, h in enumerate(heads):
                                    hb = 64 * (h % 2)
                                    nc.tensor.matmul(
                                        ps[:, hi, n0:512],
                                        vT[hb:hb + KD, hc, jc * P:(jc + 1) * P],
                                        kT[hb:hb + KD, hc, c * 512 + n0:(c + 1) * 512],
                                        start=True, stop=True)
                            et = expp.tile([P, 2, 512], f16, tag="exp")
                            # bias=-5: softmax is shift-invariant (both the
                            # attend numerator and the ones-column row-sum
                            # scale by e^-5), keeps exp within fp16 range
                            if ABLATE != "scores":
                                nc.scalar.activation(
                                    et[:, :, e0:512], ps[:, :, e0:512], AF.Exp,
                                    bias=neg5_t[:])
                                if d >= 0:
                                    # zero att where j > i (in-place triangle
                                    # select over both heads at once)
                                    nc.gpsimd.affine_select(
                                        out=et[:, :, e0:e0 + P],
                                        in_=et[:, :, e0:e0 + P],
                                        compare_op=OP.is_ge,
                                        fill=0.0, base=0,
                                        pattern=[[0, 2], [1, P]],
                                        channel_multiplier=-1)
                            else:
                                nc.gpsimd.memset(et[:, :, e0:512], 0.5)
                            tiles[jc] = et
                        exp_store[(hpair, c)] = tiles

                    def emit_attend(hpair, c):
                        heads = (2 * hpair, 2 * hpair + 1)
                        tiles = exp_store.pop((hpair, c))
                        # two b-slots share one psum bank (2 x 130 f32);
                        # normalization is batched per b-pair: one strided
                        # reciprocal + one broadcast multiply for both slots
                        pa_t = pa.tile([P, 2, 130], f32, tag="pa")
                        for b0 in range(4 * c, 4 * c + 4, 2):
                            if ABLATE == "attend":
                                for h in heads:
                                    nc.gpsimd.memset(
                                        x_new[:, b0:b0 + 2,
                                              h * 64:(h + 1) * 64], 0.1)
                                continue
                            for s, b in enumerate((b0, b0 + 1)):
                                lc0 = (b - 4 * c) * P
                                # both heads of the pair accumulate into one
                                # psum bank: head h' at cols [65*h', 65*h'+65)
                                for hi, h in enumerate(heads):
                                    for jc in range(b + 1):
                                        nc.tensor.matmul(
                                            pa_t[:, s, 65 * hi:65 * hi + 65],
                                            tiles[jc][:, hi, lc0:lc0 + P],
                                            q_sb[:, jc, h, :],
                                            start=(jc == 0), stop=(jc == b))
                            pa4 = pa_t[:].rearrange("p s (h x) -> p s h x", h=2)
                            rc = sm.tile([P, 2, 2], f32, tag="rc")
                            nc.vector.reciprocal(rc[:], pa4[:, :, :, 64])
                            # x_new[:, b0:b0+2, pair] = att_u @ q * recip
                            # (recip broadcast 64-wide per head, 0-stride)
                            xdst = x_new[:, b0:b0 + 2,
                                         hpair * P:(hpair + 1) * P].rearrange(
                                "p b (h x) -> p b h x", h=2)
                            nc.vector.tensor_tensor(
                                xdst, pa4[:, :, :, 0:64],
                                rc[:, :, :, None].to_broadcast((P, 2, 2, 64)),
                                OP.mult)

                    # schedule: kv chunk p -> attends of pair p-1 (+ their
                    # transposes) -> scores of pair p. Scores sit last in each
                    # iteration because the pp2 ring throttles them to ACT's
                    # exp pace — everything PE-independent is emitted first.
                    for p in range(H // 2):
                        emit_kv(p)
                        if p == 0:
                            emit_scores(0, 0)
                            emit_scores(0, 1)
                            emit_q2(0)
                            emit_q2(1)
                        else:
                            if p == 1:
                                emit_q2(2)
                                emit_q2(3)
                            emit_attend(p - 1, 0)
                            emit_xnewT(p - 1, 0)
                            emit_attend(p - 1, 1)
                            emit_xnewT(p - 1, 1)
                            emit_scores(p, 0)
                            emit_scores(p, 1)
                    emit_attend(3, 0)
                    emit_xnewT(3, 0)
                    emit_attend(3, 1)
                    emit_xnewT(3, 1)

                    # MLP1: h1T = relu(W1^T x_newT + b1), one [P,2,512] psum
                    # pair + one wide ACT relu per out chunk; fc outer / lc
                    # inner so consecutive matmuls share the stationary
                    if MLP_FP8:
                        h1T = t8p.tile([P, FC, L], f8, tag="t8")
                    else:
                        h1T = bigT.tile([P, FC, L], f32r, tag="bigT")
                    for oc in range(FC):
                        ps = pp2.tile([P, 2, 512], f32, tag="pp2")
                        if MLP_FP8:
                            for fc2 in range(0, FC, 2):
                                for lc in range(2):
                                    nc.tensor.matmul(
                                        ps[:, lc, :],
                                        w1_t[:, fc2:fc2 + 2, oc * P:(oc + 1) * P],
                                        x_newT[:, fc2:fc2 + 2,
                                               lc * 512:(lc + 1) * 512],
                                        start=(fc2 == 0), stop=(fc2 == FC - 2),
                                        perf_mode=DR)
                        else:
                            for fc in range(FC):
                                for lc in range(2):
                                    nc.tensor.matmul(
                                        ps[:, lc, :],
                                        w1_t[:, fc, oc * P:(oc + 1) * P],
                                        x_newT[:, fc, lc * 512:(lc + 1) * 512],
                                        start=(fc == 0), stop=(fc == FC - 1))
                        bias = b1_sb[:, li, oc:oc + 1] if use_b1 else 0.0
                        nc.scalar.activation(
                            h1T[:, oc, :], ps[:].rearrange("p a b -> p (a b)"),
                            AF.Relu, bias=bias)

                    # MLP2 + residual + LN -> y, l-blocks in pairs, with the
                    # y transposes (and, last layer, the unembed) interleaved
                    # per pair so the layer-boundary chain stays pipelined
                    if use_b2:
                        b2_b = bcast_row(b2[li], f"b2_{li}")
                    if use_gamma:
                        gamma_b = bcast_row(gamma[li], f"g_{li}")
                    if use_beta:
                        beta_b = bcast_row(beta[li], f"be_{li}")
                    if last:
                        wo = []
                        for vc in range(2):
                            wt = wp.tile([P, FC, 512], f32r, tag="w", bufs=2)
                            nc.sync.dma_start(
                                wt[:],
                                _r(Wout[:, vc * 512:(vc + 1) * 512]
                                   .rearrange("(c p) o -> p c o", p=P)))
                            wo.append(wt)
                        xT_next = bigT.tile([P, FC, L], f32r, tag="bigT")
                    else:
                        xT_next = t8p.tile([P, FC, L], f8, tag="t8")
                    y = natp.tile([P, LB, F], f32, tag="nat")
                    mv8 = sm.tile([P, LB, 2], f32, tag="mv8")
                    rstd8 = sm.tile([P, LB], f32, tag="rs8")

                    def emit_unembed(b):
                        # fc outer / vc inner: consecutive matmuls share the
                        # stationary xT block -> one ldweights per fc
                        ps = pp2.tile([P, 2, 512], f32, tag="pp2")
                        for fc in range(FC):
                            for vc in range(2):
                                nc.tensor.matmul(
                                    ps[:, vc, :],
                                    xT_next[:, fc, b * P:(b + 1) * P],
                                    wo[vc][:, fc, :],
                                    start=(fc == 0), stop=(fc == FC - 1))
                        ot = outp.tile([P, V], f32, tag="o")
                        psf = ps[:].rearrange("p a b -> p (a b)")
                        if use_bout:
                            nc.vector.tensor_add(ot[:], psf, bout_b[:])
                        else:
                            eng = (nc.vector.tensor_copy if b % 2
                                   else nc.scalar.copy)
                            eng(ot[:], psf)
                        nc.sync.dma_start(out[b * P:(b + 1) * P, :], ot[:])

                    for bp in range(LB // 2):
                        ps = pp2.tile([P, 2, 512], f32, tag="pp2")
                        for i2 in range(2):
                            b = 2 * bp + i2
                            if MLP_FP8:
                                for fc2 in range(0, FC, 2):
                                    nc.tensor.matmul(
                                        ps[:, i2, :],
                                        h1T[:, fc2:fc2 + 2, b * P:(b + 1) * P],
                                        w2_t[:, fc2:fc2 + 2, :],
                                        start=(fc2 == 0), stop=(fc2 == FC - 2),
                                        perf_mode=DR)
                            else:
                                for fc in range(FC):
                                    nc.tensor.matmul(
                                        ps[:, i2, :],
                                        h1T[:, fc, b * P:(b + 1) * P],
                                        w2_t[:, fc, :],
                                        start=(fc == 0), stop=(fc == FC - 1))
                        t2 = y[:, 2 * bp:2 * bp + 2, :]
                        nc.vector.tensor_add(t2, ps[:], x_nat[:, 2 * bp:2 * bp + 2, :])
                        if use_b2:
                            nc.vector.tensor_add(
                                t2, t2,
                                b2_b[:, None, :].to_broadcast((P, 2, F)))
                        for i2 in range(2):
                            b = 2 * bp + i2
                            st = sm.tile([P, 6], f32, tag="st")
                            nc.vector.bn_stats(st[:], y[:, b, :])
                            nc.vector.bn_aggr(mv8[:, b, :], st[:])
                            # rstd = exp(-0.5*ln(var+eps)) per chunk: the
                            # layer tail stays pipelined
                            nc.scalar.activation(
                                rstd8[:, b:b + 1], mv8[:, b, 1:2], AF.Ln,
                                bias=eps_t[:])
                            nc.scalar.activation(
                                rstd8[:, b:b + 1], rstd8[:, b:b + 1], AF.Exp,
                                scale=-0.5)
                            _ln_apply(nc, y, b, mv8, rstd8, use_gamma,
                                      use_beta,
                                      gamma_b if use_gamma else None,
                                      beta_b if use_beta else None)
                        transpose_blocks(y, xT_next, 2 * bp, 2)
                        if last:
                            emit_unembed(2 * bp)
                            emit_unembed(2 * bp + 1)

                    x_nat = y
                    xT8 = xT_next
    nc.compile()
    return nc


def _get_nc(flags, repeat=1):
    key = (flags, repeat, ABLATE, LN_BATCH, PSUM_CFG, TCOPY, EXPP_BUFS,
           DR_MODE)
    if key not in _NC_CACHE:
        _NC_CACHE[key] = _build(flags, repeat)
    return _NC_CACHE[key]


def make_runner(flags, in_maps, repeat=1):
    """Build a reusable jitted SPMD runner with device-resident inputs.

    Returns (run, split_outputs) where run() executes the kernel once on all
    8 cores and blocks; used by test.py for timing without per-call host->device
    input transfer.
    """
    import jax
    from jax.sharding import Mesh, PartitionSpec, NamedSharding
    from concourse import bass2jax, mybir as _mybir

    bass2jax.install_neuronx_cc_hook()
    nc = _get_nc(flags, repeat)
    partition_name = (nc.partition_id_tensor.name if nc.partition_id_tensor
                      else None)
    in_names, out_names, out_avals, zero_outs = [], [], [], []
    for alloc in nc.m.functions[0].allocations:
        if not isinstance(alloc, _mybir.MemoryLocationSet):
            continue
        name = alloc.memorylocations[0].name
        if alloc.kind == "ExternalInput":
            if name != partition_name:
                in_names.append(name)
        elif alloc.kind == "ExternalOutput":
            shape = tuple(alloc.tensor_shape)
            dtype = _mybir.dt.np(alloc.dtype)
            out_names.append(name)
            out_avals.append(jax.core.ShapedArray(shape, dtype))
            zero_outs.append(np.zeros(shape, dtype))
    n_params = len(in_names)
    n_outs = len(out_avals)
    all_names = in_names + out_names + ([partition_name] if partition_name else [])

    def _body(*args):
        operands = list(args)
        if partition_name is not None:
            operands.append(bass2jax.partition_id_tensor())
        outs = bass2jax._bass_exec_p.bind(
            *operands,
            out_avals=tuple(out_avals),
            in_names=tuple(all_names),
            out_names=tuple(out_names),
            lowering_input_output_aliases=(),
            sim_require_finite=True,
            sim_require_nnan=True,
            nc=nc,
        )
        return tuple(outs)

    from jax.experimental.shard_map import shard_map
    devices = jax.devices()[:NCORES]
    mesh = Mesh(np.asarray(devices), ("core",))
    in_specs = (PartitionSpec("core"),) * (n_params + n_outs)
    out_specs = (PartitionSpec("core"),) * n_outs
    sharded = jax.jit(
        shard_map(_body, mesh=mesh, in_specs=in_specs, out_specs=out_specs,
                  check_rep=False),
        keep_unused=True,
    )
    concat_in = [
        np.concatenate([np.asarray(in_maps[c][nm])[None] for c in range(NCORES)],
                       axis=0).reshape(NCORES * np.asarray(in_maps[0][nm]).shape[0],
                                       *np.asarray(in_maps[0][nm]).shape[1:])
        for nm in in_names
    ]
    sh = NamedSharding(mesh, PartitionSpec("core"))
    dev_in = [jax.device_put(x, sh) for x in concat_in]
    dev_zeros = [
        jax.device_put(np.zeros((NCORES * z.shape[0], *z.shape[1:]), z.dtype), sh)
        for z in zero_outs
    ]

    def run():
        outs = sharded(*dev_in, *dev_zeros)
        jax.block_until_ready(outs)
        return outs

    def split(outs):
        return [
            {nm: np.asarray(outs[i]).reshape(NCORES, *out_avals[i].shape)[c]
             for i, nm in enumerate(out_names)}
            for c in range(NCORES)
        ]

    return run, split


def prep_args(inputs):
    """Host-side arg prep shared by kernel() and test.py: fp32 copies of the
    fp32 params, fp8e4m3 casts of Wq/Wk/Wv (consumed by the DoubleRow
    projection matmuls)."""
    import ml_dtypes
    args = {k: np.ascontiguousarray(np.asarray(v), dtype=np.float32)
            for k, v in inputs.items() if k not in ("tokens", "Wq", "Wk", "Wv")}
    for k in ("Wq", "Wk", "Wv"):
        args[k + "8"] = np.ascontiguousarray(
            np.asarray(inputs[k], dtype=np.float32).astype(ml_dtypes.float8_e4m3))
    if MLP_FP8:
        for k in ("W1", "W2"):
            args[k + "8"] = np.ascontiguousarray(
                args[k].astype(ml_dtypes.float8_e4m3))
    return args


def kernel(**inputs) -> np.ndarray:
    tokens = np.asarray(inputs["tokens"])
    args = prep_args(inputs)
    flags = (
        bool(np.any(args["b1"])),
        bool(np.any(args["b2"])),
        bool(np.any(args["gamma"] != 1.0)),
        bool(np.any(args["beta"])),
        bool(np.any(args["bout"])),
    )
    nc = _get_nc(flags)
    tok32 = np.ascontiguousarray(tokens.astype(np.int32))
    in_maps = [dict(args, tokens=tok32[c]) for c in range(NCORES)]
    res = run_bass_kernel_spmd(nc, in_maps, list(range(NCORES)))
    return np.stack([res.results[c]["out"] for c in range(NCORES)], axis=0)


if __name__ == "__main__":
    rng = np.random.default_rng(0)
    toy = {
        "tokens": rng.integers(0, V, size=(N, L)),
        "embed": rng.standard_normal((V, F)).astype(np.float32) * 0.02,
        "Wq": rng.standard_normal((NL, F, HQ)).astype(np.float32) * 0.02,
        "Wk": rng.standard_normal((NL, F, H * KD)).astype(np.float32) * 0.02,
        "Wv": rng.standard_normal((NL, F, H * KD)).astype(np.float32) * 0.02,
        "W1": rng.standard_normal((NL, HQ, F)).astype(np.float32) * 0.02,
        "b1": np.zeros((NL, F), np.float32),
        "W2": rng.standard_normal((NL, F, F)).astype(np.float32) * 0.02,
        "b2": np.zeros((NL, F), np.float32),
        "gamma": np.ones((NL, F), np.float32),
        "beta": np.zeros((NL, F), np.float32),
        "Wout": rng.standard_normal((F, V)).astype(np.float32) * 0.02,
        "bout": np.zeros((V,), np.float32),
    }
    o = kernel(**toy)
    print("out:", o.shape, o.dtype, float(np.abs(o).max()))


# revision 56
# speedup vs baseline: 1.2735x; 1.0617x over previous
"""Trainium2 Bass kernel for nn_AttentionModel (4-layer dense transformer).

Contract: kernel(**inputs) takes FULL unsharded inputs (as produced by
setup_inputs) and returns the FULL output [N, L, V] fp32.

Sharding: data-parallel over batch N=8 across the 8 NeuronCores — each core
runs the complete transformer for one batch element (identical NEFF, per-core
tokens). No collectives needed; the host stacks the per-core outputs.

Per-core dataflow (L=1024, F=512, H=8, KD=QD=64, NL=4, V=1024):
  - embedding: indirect-DMA gather of embed rows by token -> x0 natural [L, F]
  - activations kept in two layouts:
      natural [l(128-part) x F]  - for layernorm / residual / softmax scales
      T       [F(128-part) x L]  - as matmul operands (contraction on
      partitions); the T copy consumed by Q/K/V is fp8e4m3 (xT8), the one
      consumed by the unembed stays f32r. PE-transposes convert layouts.
  - per layer (emission interleaved for cross-engine overlap — see the
    schedule comment in the layer loop):
      kT = Wk^T x^T, vT = Wv^T x^T: fp8 DoubleRow matmuls (two 256-deep
           K-passes instead of four 128-deep f32r passes, 4x fewer PE
           cycles); psum pair-tile -> one f32r copy per output chunk
      q  = x Wq fp8 DoubleRow, stored fp16 as [j-chunk, head, 65] with a
           ones column so the attend matmul also produces softmax row-sums
      scores^T[j,i] = v k^T per head in f32r (K=64 matmuls on disjoint PE
           row-groups per head pair; causal tiles only)
      att_u = exp(scores^T - 5) in fp16: both heads of a pair share one
           2-bank psum tile so exp runs as a single wide ACT instruction;
           diagonal tiles triangle-zeroed in place with one gpsimd
           affine_select per pair (keep j<=i)
      x_new[i-block, pair] = att_u^T @ [q | 1] (fp16 matmuls, one psum bank
           per pair): col 64 of each head = softmax row-sum; strided
           reciprocal + 0-stride-broadcast multiply normalize on DVE
      x_newT via PE transposes (4 outputs share one psum bank -> one merged
           copy); MLP h1T = relu(W1^T x_newT + b1) (f32r, ACT relu+bias,
           one [P,2,512] psum pair per output chunk); h = h1T^T W2 (f32r,
           l-block pairs); y = LN(x + h) (bn_stats/bn_aggr on DVE, rstd on
           ACT, apply on gpsimd); yT via PE transposes -> xT8 (fp8) or xTr
           (f32r, last layer, feeds unembed)
  - unembed: logits = x4 Wout + bout in f32r, one [128, 1024] DMA per block.

Engine budget notes: gpsimd (Pool) cannot touch PSUM on TRN2, so all
psum->sbuf traffic is on DVE/ACT (split via TCOPY) and Pool takes the
SBUF-only work (affine_select, LN apply). The ACT table-set choice is pinned
(see _Bacc) so Exp/Ln/Relu/Copy share one loaded set - no per-layer ~2.7us
table swaps. fp8 is limited to the Q/K/V projections: k/v quantization noise
is washed out by the softmax ratio, q noise by attention averaging; scores,
attend, MLP and unembed keep their f32r/fp16 envelopes (measured end-to-end
rel err vs fp32 reference ~3e-3, budget 2e-2).
"""

import numpy as np

import concourse.bass as bass
import concourse.mybir as mybir
import concourse.tile as tile
from concourse import bacc
from concourse.bass_utils import run_bass_kernel_spmd
from concourse.masks import make_identity

# Model dims (hardcoded per the problem spec)
V, F, NL, H, KD, QD = 1024, 512, 4, 8, 64, 64
N, L = 8, 1024
HQ = H * QD  # 512
P = 128
FC = F // P      # 4 f-chunks
LB = L // P      # 8 l-blocks of 128
NCORES = 8

f32 = mybir.dt.float32
f32r = mybir.dt.float32r
f16 = mybir.dt.float16
f8 = mybir.dt.float8e4
i32 = mybir.dt.int32
AF = mybir.ActivationFunctionType
OP = mybir.AluOpType
DR = mybir.MatmulPerfMode.DoubleRow

_NC_CACHE: dict = {}
ABLATE = "none"  # perf-analysis knob: none|scores|attend|transposes
DR_MODE = "dr"  # q/k/v projection matmul mode: dr (fp8 DoubleRow, 256-deep
# K per pass) | fp8 (plain fp8, 128-deep chunks — isolates DoubleRow's real
# hw throughput from the fp8 layout changes)
MLP_FP8 = False  # MLP1/MLP2 in fp8 DoubleRow (x_newT/h1T stored fp8);
# False keeps the f32r MLP path (measured faster on hw in-process A/B and
# halves the end-to-end error: 5.5e-3 vs 1.1e-2)
LN_BATCH = False  # batch the LN ln/exp across the 8 l-chunks
TCOPY = "dve"  # engine for merged y/x0 transpose copies: dve|act|split
# (674us vs 683us for act in same-process hw A/B)
EXPP_BUFS = 22  # in-flight fp16 att PAIR tiles ([P,2,512]); the interleaved
# schedule keeps pair p's 12 tiles live while pair p+1's 12 are produced
PSUM_CFG = (3, 1, 1)  # bufs for (pp2, pa, pt). pp2 tiles are [P,2,512]
# (2 banks, shared by scores pairs / projection pairs / mlp / unembed); pa
# packs 2 attend accumulators of 130 f32 into one bank; pt packs 4 transpose
# outputs into one bank. Banks: 3*2 + 1 + 1 = 8.


class _Bacc(bacc.Bacc):
    """Bacc with activation-table-set selection pinned to
    natural_log_exp_and_others (contains Exp, Ln, Relu, Copy — everything this
    kernel uses) so the load-insertion pass emits one table load instead of
    thrashing between per-function sets (~2.7us per swap)."""

    def insert_act_table_loads(self):
        from concourse.hw_specs import get_activation_tables
        import concourse.mybir as _mb

        has_activation = any(
            isinstance(i, _mb.InstActivation)
            for b in self.main_func.blocks
            for i in b.instructions
        )
        if not has_activation:
            return
        keep = {AF.Exp, AF.Ln, AF.Relu, AF.Copy}
        chosen = "natural_log_exp_and_others"
        full = get_activation_tables(self.m.arch)
        assert keep <= full[chosen], (chosen, keep - full[chosen])
        tables = [
            (name, (fns if name == chosen else fns - keep))
            for name, fns in full.items()
        ]
        import bass_rust as _bass_rust
        _bass_rust.insert_act_table_loads(self, tables)


def _ln_apply(nc, y, b, mv8, rstd8, use_gamma, use_beta, gamma_b, beta_b):
    t = y[:, b, :]
    # SBUF in-place: runs on gpsimd to keep DVE free for psum traffic
    nc.gpsimd.tensor_scalar(
        t, t, mv8[:, b, 0:1], rstd8[:, b:b + 1],
        op0=OP.subtract, op1=OP.mult)
    if use_gamma:
        nc.gpsimd.tensor_mul(t, t, gamma_b[:])
    if use_beta:
        nc.gpsimd.tensor_add(t, t, beta_b[:])


def _r(ap):
    """View a DRAM fp32 AP as float32r for DMA into f32r tiles."""
    return ap.bitcast(f32r)


def _build(flags, repeat=1):
    use_b1, use_b2, use_gamma, use_beta, use_bout = flags
    nc = _Bacc("TRN2", target_bir_lowering=False, debug=False,
               num_devices=NCORES)

    tokens = nc.declare_dram_parameter("tokens", [L], i32, isOutput=False)
    embed = nc.declare_dram_parameter("embed", [V, F], f32, isOutput=False)
    Wq8 = nc.declare_dram_parameter("Wq8", [NL, F, HQ], f8, isOutput=False)
    Wk8 = nc.declare_dram_parameter("Wk8", [NL, F, H * KD], f8, isOutput=False)
    Wv8 = nc.declare_dram_parameter("Wv8", [NL, F, H * KD], f8, isOutput=False)
    if MLP_FP8:
        W18 = nc.declare_dram_parameter("W18", [NL, HQ, F], f8, isOutput=False)
        W28 = nc.declare_dram_parameter("W28", [NL, F, F], f8, isOutput=False)
    W1 = nc.declare_dram_parameter("W1", [NL, HQ, F], f32, isOutput=False)
    b1 = nc.declare_dram_parameter("b1", [NL, F], f32, isOutput=False)
    W2 = nc.declare_dram_parameter("W2", [NL, F, F], f32, isOutput=False)
    b2 = nc.declare_dram_parameter("b2", [NL, F], f32, isOutput=False)
    gamma = nc.declare_dram_parameter("gamma", [NL, F], f32, isOutput=False)
    beta = nc.declare_dram_parameter("beta", [NL, F], f32, isOutput=False)
    Wout = nc.declare_dram_parameter("Wout", [F, V], f32, isOutput=False)
    bout = nc.declare_dram_parameter("bout", [V], f32, isOutput=False)
    out = nc.declare_dram_parameter("out", [L, V], f32, isOutput=True)

    with tile.TileContext(nc) as tc:
        with (
            tc.tile_pool(name="bigT", bufs=3) as bigT,    # [P, FC, L] f32r
            tc.tile_pool(name="t8", bufs=4 if MLP_FP8 else 2) as t8p,       # [P, FC, L] fp8
            tc.tile_pool(name="nat", bufs=3) as natp,     # [P, LB, F] f32
            tc.tile_pool(name="qp", bufs=1) as qp,        # [P, LB, H, 65] f16
            tc.tile_pool(name="expp", bufs=EXPP_BUFS) as expp,  # [P, 2, 512] f16
            tc.tile_pool(name="wp", bufs=4) as wp,
            tc.tile_pool(name="cst", bufs=1) as cst,
            tc.tile_pool(name="sm", bufs=16) as sm,       # small per-partition scalars
            tc.tile_pool(name="op", bufs=2) as outp,      # [P, 1024] out staging
            tc.tile_pool(name="pp2", bufs=PSUM_CFG[0], space="PSUM") as pp2,
            tc.tile_pool(name="pa", bufs=PSUM_CFG[1], space="PSUM") as pa,
            tc.tile_pool(name="pt", bufs=PSUM_CFG[2], space="PSUM") as pt,
        ):
            # ---- constants ----
            ident = cst.tile([P, P], f32, tag="ident")
            make_identity(nc, ident[:])
            eps_t = cst.tile([P, 1], f32, tag="eps")
            nc.vector.memset(eps_t[:], 1e-5)
            neg5_t = cst.tile([P, 1], f32, tag="neg5")
            nc.vector.memset(neg5_t[:], -5.0)
            if use_b1:
                b1_sb = cst.tile([P, NL, FC], f32, tag="b1")
                nc.sync.dma_start(b1_sb[:], b1.rearrange("l (c p) -> p l c", p=P))
            if use_bout:
                bout_b = cst.tile([P, V], f32, tag="bout")
                bout_ap = bout[:]
                nc.sync.dma_start(
                    bout_b[:],
                    bass.AP(tensor=bout_ap.tensor, offset=bout_ap.offset,
                            ap=[[0, P]] + bout_ap.ap),
                )

            def bcast_row(dram_row_ap, tag):
                t = cst.tile([P, F], f32, tag=tag)
                nc.sync.dma_start(
                    t[:],
                    bass.AP(tensor=dram_row_ap.tensor, offset=dram_row_ap.offset,
                            ap=[[0, P]] + dram_row_ap.ap),
                )
                return t

            import contextlib
            _loop = (tc.For_i(0, repeat, 1) if repeat > 1
                     else contextlib.nullcontext())
            with _loop:
                # ---- embedding gather ----
                tok_sb = cst.tile([P, LB], i32, tag="tok")
                nc.sync.dma_start(tok_sb[:], tokens.rearrange("(b p) -> p b", p=P))
                x_nat = natp.tile([P, LB, F], f32, tag="nat")
                for b in range(LB):
                    nc.gpsimd.indirect_dma_start(
                        out=x_nat[:, b, :], out_offset=None,
                        in_=embed[:],
                        in_offset=bass.IndirectOffsetOnAxis(ap=tok_sb[:, b:b + 1], axis=0),
                    )

                def tcopy(i, dst, src):
                    """Merged psum->sbuf copy; engine per TCOPY."""
                    if TCOPY == "dve" or (TCOPY == "split" and i % 2 == 0):
                        nc.vector.tensor_copy(dst, src)
                    else:
                        nc.scalar.copy(dst, src)

                def transpose_blocks(src_nat, dst_T, b0, nb):
                    """Transpose l-blocks [b0, b0+nb) of natural [P, LB, F]
                    f32 into T layout [P, FC, L] (dtype cast per dst tile:
                    f32r or fp8). 4 transpose outputs share one psum bank and
                    drain with a single merged copy."""
                    if ABLATE == "transposes":
                        nc.gpsimd.memset(dst_T[:, :, b0 * P:(b0 + nb) * P], 0.1)
                        return
                    for b in range(b0, b0 + nb):
                        pt_ps = pt.tile([P, FC, P], f32, tag="pt")
                        for c in range(FC):
                            nc.tensor.transpose(
                                pt_ps[:, c, :],
                                src_nat[:, b, c * P:(c + 1) * P], ident[:])
                        tcopy(b, dst_T[:, :, b * P:(b + 1) * P], pt_ps[:])

                xT8 = t8p.tile([P, FC, L], f8, tag="t8")
                transpose_blocks(x_nat, xT8, 0, LB)

                # ---- layers ----
                for li in range(NL):
                    last = li == NL - 1
                    wq8_t = wp.tile([P, FC, HQ], f8, tag="w8", bufs=5 if MLP_FP8 else 3)
                    wk8_t = wp.tile([P, FC, HQ], f8, tag="w8", bufs=5 if MLP_FP8 else 3)
                    wv8_t = wp.tile([P, FC, HQ], f8, tag="w8", bufs=5 if MLP_FP8 else 3)
                    nc.sync.dma_start(wq8_t[:], Wq8[li].rearrange("(c p) o -> p c o", p=P))
                    nc.sync.dma_start(wk8_t[:], Wk8[li].rearrange("(c p) o -> p c o", p=P))
                    nc.sync.dma_start(wv8_t[:], Wv8[li].rearrange("(c p) o -> p c o", p=P))
                    if MLP_FP8:
                        w1_t = wp.tile([P, FC, F], f8, tag="w8", bufs=5 if MLP_FP8 else 3)
                        w2_t = wp.tile([P, FC, F], f8, tag="w8", bufs=5 if MLP_FP8 else 3)
                        nc.sync.dma_start(
                            w1_t[:], W18[li].rearrange("(c p) o -> p c o", p=P))
                        nc.sync.dma_start(
                            w2_t[:], W28[li].rearrange("(c p) o -> p c o", p=P))
                    else:
                        w1_t = wp.tile([P, FC, F], f32r, tag="w", bufs=4)
                        w2_t = wp.tile([P, FC, F], f32r, tag="w", bufs=4)
                        nc.sync.dma_start(
                            w1_t[:], _r(W1[li].rearrange("(c p) o -> p c o", p=P)))
                        nc.sync.dma_start(
                            w2_t[:], _r(W2[li].rearrange("(c p) o -> p c o", p=P)))

                    # kT, vT chunk oc: fp8 DoubleRow (256-deep K per pass),
                    # one [P,2,512] psum pair -> one 1024-wide f32r copy
                    kT = bigT.tile([P, FC, L], f32r, tag="bigT")
                    vT = bigT.tile([P, FC, L], f32r, tag="bigT")

                    def emit_kv(oc):
                        for wi, (w8_t, oT) in enumerate(((wk8_t, kT), (wv8_t, vT))):
                            ps = pp2.tile([P, 2, 512], f32, tag="pp2")
                            if DR_MODE == "dr":
                                # fc2 outer / lc inner: consecutive matmuls
                                # share the stationary -> one ldweights per
                                # weight block
                                for fc2 in range(0, FC, 2):
                                    for lc in range(2):
                                        nc.tensor.matmul(
                                            ps[:, lc, :],
                                            w8_t[:, fc2:fc2 + 2, oc * P:(oc + 1) * P],
                                            xT8[:, fc2:fc2 + 2, lc * 512:(lc + 1) * 512],
                                            start=(fc2 == 0), stop=(fc2 == FC - 2),
                                            perf_mode=DR)
                            else:
                                for fc in range(FC):
                                    for lc in range(2):
                                        nc.tensor.matmul(
                                            ps[:, lc, :],
                                            w8_t[:, fc, oc * P:(oc + 1) * P],
                                            xT8[:, fc, lc * 512:(lc + 1) * 512],
                                            start=(fc == 0), stop=(fc == FC - 1))
                            # chunk 0 drains on ACT: it gates the layer's
                            # first scores while DVE still holds the previous
                            # layer's transpose-copy backlog
                            eng = (nc.scalar.copy if oc == 0
                                   else nc.vector.tensor_copy)
                            eng(oT[:, oc, :],
                                ps[:].rearrange("p a b -> p (a b)"))

                    # q natural (fp16 for the attend matmul), [P(j), jc, head, 65]
                    # with a trailing ones column so attend also yields row-sums
                    q_sb = qp.tile([P, LB, H, 65], f16, tag="q")
                    nc.vector.memset(q_sb[:, :, :, 64:65], 1.0)

                    def emit_q2(bp):
                        ps = pp2.tile([P, 2, 512], f32, tag="pp2")
                        for i2 in range(2):
                            b = 2 * bp + i2
                            if DR_MODE == "dr":
                                for fc2 in range(0, FC, 2):
                                    nc.tensor.matmul(
                                        ps[:, i2, :],
                                        xT8[:, fc2:fc2 + 2, b * P:(b + 1) * P],
                                        wq8_t[:, fc2:fc2 + 2, :],
                                        start=(fc2 == 0), stop=(fc2 == FC - 2),
                                        perf_mode=DR)
                            else:
                                for fc in range(FC):
                                    nc.tensor.matmul(
                                        ps[:, i2, :],
                                        xT8[:, fc, b * P:(b + 1) * P],
                                        wq8_t[:, fc, :],
                                        start=(fc == 0), stop=(fc == FC - 1))
                        eng = nc.vector.tensor_copy if bp % 2 else nc.scalar.copy
                        eng(q_sb[:, 2 * bp:2 * bp + 2, :, 0:64],
                            ps[:].rearrange("p b (h d) -> p b h d", h=H))

                    x_new = natp.tile([P, LB, F], f32, tag="nat")
                    if MLP_FP8:
                        x_newT = t8p.tile([P, FC, L], f8, tag="t8")
                    else:
                        x_newT = bigT.tile([P, FC, L], f32r, tag="bigT")
                    exp_store: dict = {}

                    def emit_xnewT(p, half):
                        # transposes of x_new chunk p (head pair p's columns)
                        # for the 4 l-blocks finished by attend(p, half).
                        # Pair 3's copies gate MLP1 and run when the exps are
                        # done, so they drain on the then-idle ACT.
                        pt_ps = pt.tile([P, 4, P], f32, tag="pt")
                        for i, b in enumerate(range(4 * half, 4 * half + 4)):
                            nc.tensor.transpose(
                                pt_ps[:, i, :],
                                x_new[:, b, p * P:(p + 1) * P], ident[:])
                        eng = nc.scalar.copy if p == 3 else nc.vector.tensor_copy
                        eng(x_newT[:, p, 4 * half * P:(4 * half + 4) * P],
                            pt_ps[:])

                    def emit_scores(hpair, c):
                        heads = (2 * hpair, 2 * hpair + 1)
                        hc = hpair
                        tiles = {}
                        for jc in range(4 * c + 4):
                            d = jc - 4 * c
                            n0 = 0 if d < 0 else min(P * d, 256)
                            e0 = 0 if d < 0 else P * d
                            # both heads of the pair in one 2-bank psum tile
                            # so exp / affine_select run as single wide
                            # instructions (halves the per-instr ACT access
                            # latency spend)
                            ps = pp2.tile([P, 2, 512], f32, tag="pp2")
                            if ABLATE != "scores":
                                for hi, h in enumerate(heads):
                                    hb = 64 * (h % 2)
                                    nc.tensor.matmul(
                                        ps[:, hi, n0:512],
                                        vT[hb:hb + KD, hc, jc * P:(jc + 1) * P],
                                        kT[hb:hb + KD, hc, c * 512 + n0:(c + 1) * 512],
                                        start=True, stop=True)
                            et = expp.tile([P, 2, 512], f16, tag="exp")
                            # bias=-5: softmax is shift-invariant (both the
                            # attend numerator and the ones-column row-sum
                            # scale by e^-5), keeps exp within fp16 range
                            if ABLATE != "scores":
                                nc.scalar.activation(
                                    et[:, :, e0:512], ps[:, :, e0:512], AF.Exp,
                                    bias=neg5_t[:])
                                if d >= 0:
                                    # zero att where j > i (in-place triangle
                                    # select over both heads at once)
                                    nc.gpsimd.affine_select(
                                        out=et[:, :, e0:e0 + P],
                                        in_=et[:, :, e0:e0 + P],
                                        compare_op=OP.is_ge,
                                        fill=0.0, base=0,
                                        pattern=[[0, 2], [1, P]],
                                        channel_multiplier=-1)
                            else:
                                nc.gpsimd.memset(et[:, :, e0:512], 0.5)
                            tiles[jc] = et
                        exp_store[(hpair, c)] = tiles

                    def emit_attend(hpair, c):
                        heads = (2 * hpair, 2 * hpair + 1)
                        tiles = exp_store.pop((hpair, c))
                        # two b-slots share one psum bank (2 x 130 f32);
                        # normalization is batched per b-pair: one strided
                        # reciprocal + one broadcast multiply for both slots
                        pa_t = pa.tile([P, 2, 130], f32, tag="pa")
                        for b0 in range(4 * c, 4 * c + 4, 2):
                            if ABLATE == "attend":
                                for h in heads:
                                    nc.gpsimd.memset(
                                        x_new[:, b0:b0 + 2,
                                              h * 64:(h + 1) * 64], 0.1)
                                continue
                            for s, b in enumerate((b0, b0 + 1)):
                                lc0 = (b - 4 * c) * P
                                # both heads of the pair accumulate into one
                                # psum bank: head h' at cols [65*h', 65*h'+65)
                                for hi, h in enumerate(heads):
                                    for jc in range(b + 1):
                                        nc.tensor.matmul(
                                            pa_t[:, s, 65 * hi:65 * hi + 65],
                                            tiles[jc][:, hi, lc0:lc0 + P],
                                            q_sb[:, jc, h, :],
                                            start=(jc == 0), stop=(jc == b))
                            pa4 = pa_t[:].rearrange("p s (h x) -> p s h x", h=2)
                            rc = sm.tile([P, 2, 2], f32, tag="rc")
                            nc.vector.reciprocal(rc[:], pa4[:, :, :, 64])
                            # x_new[:, b0:b0+2, pair] = att_u @ q * recip
                            # (recip broadcast 64-wide per head, 0-stride)
                            xdst = x_new[:, b0:b0 + 2,
                                         hpair * P:(hpair + 1) * P].rearrange(
                                "p b (h x) -> p b h x", h=2)
                            nc.vector.tensor_tensor(
                                xdst, pa4[:, :, :, 0:64],
                                rc[:, :, :, None].to_broadcast((P, 2, 2, 64)),
                                OP.mult)

                    # schedule: kv chunk p -> attends of pair p-1 (+ their
                    # transposes) -> scores of pair p. Scores sit last in each
                    # iteration because the pp2 ring throttles them to ACT's
                    # exp pace — everything PE-independent is emitted first.
                    for p in range(H // 2):
                        emit_kv(p)
                        if p == 0:
                            emit_scores(0, 0)
                            emit_scores(0, 1)
                            emit_q2(0)
                            emit_q2(1)
                        else:
                            if p == 1:
                                emit_q2(2)
                                emit_q2(3)
                            emit_attend(p - 1, 0)
                            emit_xnewT(p - 1, 0)
                            emit_attend(p - 1, 1)
                            emit_xnewT(p - 1, 1)
                            emit_scores(p, 0)
                            emit_scores(p, 1)
                    emit_attend(3, 0)
                    emit_xnewT(3, 0)
                    emit_attend(3, 1)
                    emit_xnewT(3, 1)

                    # MLP1: h1T = relu(W1^T x_newT + b1), one [P,2,512] psum
                    # pair + one wide ACT relu per out chunk; fc outer / lc
                    # inner so consecutive matmuls share the stationary
                    if MLP_FP8:
                        h1T = t8p.tile([P, FC, L], f8, tag="t8")
                    else:
                        h1T = bigT.tile([P, FC, L], f32r, tag="bigT")
                    for oc in range(FC):
                        ps = pp2.tile([P, 2, 512], f32, tag="pp2")
                        if MLP_FP8:
                            for fc2 in range(0, FC, 2):
                                for lc in range(2):
                                    nc.tensor.matmul(
                                        ps[:, lc, :],
                                        w1_t[:, fc2:fc2 + 2, oc * P:(oc + 1) * P],
                                        x_newT[:, fc2:fc2 + 2,
                                               lc * 512:(lc + 1) * 512],
                                        start=(fc2 == 0), stop=(fc2 == FC - 2),
                                        perf_mode=DR)
                        else:
                            for fc in range(FC):
                                for lc in range(2):
                                    nc.tensor.matmul(
                                        ps[:, lc, :],
                                        w1_t[:, fc, oc * P:(oc + 1) * P],
                                        x_newT[:, fc, lc * 512:(lc + 1) * 512],
                                        start=(fc == 0), stop=(fc == FC - 1))
                        bias = b1_sb[:, li, oc:oc + 1] if use_b1 else 0.0
                        nc.scalar.activation(
                            h1T[:, oc, :], ps[:].rearrange("p a b -> p (a b)"),
                            AF.Relu, bias=bias)

                    # MLP2 + residual + LN -> y, l-blocks in pairs, with the
                    # y transposes (and, last layer, the unembed) interleaved
                    # per pair so the layer-boundary chain stays pipelined
                    if use_b2:
                        b2_b = bcast_row(b2[li], f"b2_{li}")
                    if use_gamma:
                        gamma_b = bcast_row(gamma[li], f"g_{li}")
                    if use_beta:
                        beta_b = bcast_row(beta[li], f"be_{li}")
                    if last:
                        wo = []
                        for vc in range(2):
                            wt = wp.tile([P, FC, 512], f32r, tag="w", bufs=4)
                            nc.sync.dma_start(
                                wt[:],
                                _r(Wout[:, vc * 512:(vc + 1) * 512]
                                   .rearrange("(c p) o -> p c o", p=P)))
                            wo.append(wt)
                        xT_next = bigT.tile([P, FC, L], f32r, tag="bigT")
                    else:
                        xT_next = t8p.tile([P, FC, L], f8, tag="t8")
                    y = natp.tile([P, LB, F], f32, tag="nat")
                    mv8 = sm.tile([P, LB, 2], f32, tag="mv8")
                    rstd8 = sm.tile([P, LB], f32, tag="rs8")

                    def emit_unembed(b):
                        # fc outer / vc inner: consecutive matmuls share the
                        # stationary xT block -> one ldweights per fc
                        ps = pp2.tile([P, 2, 512], f32, tag="pp2")
                        for fc in range(FC):
                            for vc in range(2):
                                nc.tensor.matmul(
                                    ps[:, vc, :],
                                    xT_next[:, fc, b * P:(b + 1) * P],
                                    wo[vc][:, fc, :],
                                    start=(fc == 0), stop=(fc == FC - 1))
                        ot = outp.tile([P, V], f32, tag="o")
                        psf = ps[:].rearrange("p a b -> p (a b)")
                        if use_bout:
                            nc.vector.tensor_add(ot[:], psf, bout_b[:])
                        else:
                            eng = (nc.vector.tensor_copy if b % 2
                                   else nc.scalar.copy)
                            eng(ot[:], psf)
                        nc.sync.dma_start(out[b * P:(b + 1) * P, :], ot[:])

                    for bp in range(LB // 2):
                        ps = pp2.tile([P, 2, 512], f32, tag="pp2")
                        for i2 in range(2):
                            b = 2 * bp + i2
                            if MLP_FP8:
                                for fc2 in range(0, FC, 2):
                                    nc.tensor.matmul(
                                        ps[:, i2, :],
                                        h1T[:, fc2:fc2 + 2, b * P:(b + 1) * P],
                                        w2_t[:, fc2:fc2 + 2, :],
                                        start=(fc2 == 0), stop=(fc2 == FC - 2),
                                        perf_mode=DR)
                            else:
                                for fc in range(FC):
                                    nc.tensor.matmul(
                                        ps[:, i2, :],
                                        h1T[:, fc, b * P:(b + 1) * P],
                                        w2_t[:, fc, :],
                                        start=(fc == 0), stop=(fc == FC - 1))
                        t2 = y[:, 2 * bp:2 * bp + 2, :]
                        nc.vector.tensor_add(t2, ps[:], x_nat[:, 2 * bp:2 * bp + 2, :])
                        if use_b2:
                            nc.vector.tensor_add(
                                t2, t2,
                                b2_b[:, None, :].to_broadcast((P, 2, F)))
                        for i2 in range(2):
                            b = 2 * bp + i2
                            st = sm.tile([P, 6], f32, tag="st")
                            nc.vector.bn_stats(st[:], y[:, b, :])
                            nc.vector.bn_aggr(mv8[:, b, :], st[:])
                            # rstd = exp(-0.5*ln(var+eps)) per chunk: the
                            # layer tail stays pipelined
                            nc.scalar.activation(
                                rstd8[:, b:b + 1], mv8[:, b, 1:2], AF.Ln,
                                bias=eps_t[:])
                            nc.scalar.activation(
                                rstd8[:, b:b + 1], rstd8[:, b:b + 1], AF.Exp,
                                scale=-0.5)
                            _ln_apply(nc, y, b, mv8, rstd8, use_gamma,
                                      use_beta,
                                      gamma_b if use_gamma else None,
                                      beta_b if use_beta else None)
                        transpose_blocks(y, xT_next, 2 * bp, 2)
                        if last:
                            emit_unembed(2 * bp)
                            emit_unembed(2 * bp + 1)

                    x_nat = y
                    xT8 = xT_next
    nc.compile()
    return nc


def _get_nc(flags, repeat=1):
    key = (flags, repeat, ABLATE, LN_BATCH, PSUM_CFG, TCOPY, EXPP_BUFS,
           DR_MODE)
    if key not in _NC_CACHE:
        _NC_CACHE[key] = _build(flags, repeat)
    return _NC_CACHE[key]


def make_runner(flags, in_maps, repeat=1):
    """Build a reusable jitted SPMD runner with device-resident inputs.

    Returns (run, split_outputs) where run() executes the kernel once on all
    8 cores and blocks; used by test.py for timing without per-call host->device
    input transfer.
    """
    import jax
    from jax.sharding import Mesh, PartitionSpec, NamedSharding
    from concourse import bass2jax, mybir as _mybir

    bass2jax.install_neuronx_cc_hook()
    nc = _get_nc(flags, repeat)
    partition_name = (nc.partition_id_tensor.name if nc.partition_id_tensor
                      else None)
    in_names, out_names, out_avals, zero_outs = [], [], [], []
    for alloc in nc.m.functions[0].allocations:
        if not isinstance(alloc, _mybir.MemoryLocationSet):
            continue
        name = alloc.memorylocations[0].name
        if alloc.kind == "ExternalInput":
            if name != partition_name:
                in_names.append(name)
        elif alloc.kind == "ExternalOutput":
            shape = tuple(alloc.tensor_shape)
            dtype = _mybir.dt.np(alloc.dtype)
            out_names.append(name)
            out_avals.append(jax.core.ShapedArray(shape, dtype))
            zero_outs.append(np.zeros(shape, dtype))
    n_params = len(in_names)
    n_outs = len(out_avals)
    all_names = in_names + out_names + ([partition_name] if partition_name else [])

    def _body(*args):
        operands = list(args)
        if partition_name is not None:
            operands.append(bass2jax.partition_id_tensor())
        outs = bass2jax._bass_exec_p.bind(
            *operands,
            out_avals=tuple(out_avals),
            in_names=tuple(all_names),
            out_names=tuple(out_names),
            lowering_input_output_aliases=(),
            sim_require_finite=True,
            sim_require_nnan=True,
            nc=nc,
        )
        return tuple(outs)

    from jax.experimental.shard_map import shard_map
    devices = jax.devices()[:NCORES]
    mesh = Mesh(np.asarray(devices), ("core",))
    in_specs = (PartitionSpec("core"),) * (n_params + n_outs)
    out_specs = (PartitionSpec("core"),) * n_outs
    sharded = jax.jit(
        shard_map(_body, mesh=mesh, in_specs=in_specs, out_specs=out_specs,
                  check_rep=False),
        keep_unused=True,
    )
    concat_in = [
        np.concatenate([np.asarray(in_maps[c][nm])[None] for c in range(NCORES)],
                       axis=0).reshape(NCORES * np.asarray(in_maps[0][nm]).shape[0],
                                       *np.asarray(in_maps[0][nm]).shape[1:])
        for nm in in_names
    ]
    sh = NamedSharding(mesh, PartitionSpec("core"))
    dev_in = [jax.device_put(x, sh) for x in concat_in]
    dev_zeros = [
        jax.device_put(np.zeros((NCORES * z.shape[0], *z.shape[1:]), z.dtype), sh)
        for z in zero_outs
    ]

    def run():
        outs = sharded(*dev_in, *dev_zeros)
        jax.block_until_ready(outs)
        return outs

    def split(outs):
        return [
            {nm: np.asarray(outs[i]).reshape(NCORES, *out_avals[i].shape)[c]
             for i, nm in enumerate(out_names)}
            for c in range(NCORES)
        ]

    return run, split


def prep_args(inputs):
    """Host-side arg prep shared by kernel() and test.py: fp32 copies of the
    fp32 params, fp8e4m3 casts of Wq/Wk/Wv (consumed by the DoubleRow
    projection matmuls)."""
    import ml_dtypes
    args = {k: np.ascontiguousarray(np.asarray(v), dtype=np.float32)
            for k, v in inputs.items() if k not in ("tokens", "Wq", "Wk", "Wv")}
    for k in ("Wq", "Wk", "Wv"):
        args[k + "8"] = np.ascontiguousarray(
            np.asarray(inputs[k], dtype=np.float32).astype(ml_dtypes.float8_e4m3))
    if MLP_FP8:
        for k in ("W1", "W2"):
            args[k + "8"] = np.ascontiguousarray(
                args[k].astype(ml_dtypes.float8_e4m3))
    return args


def kernel(**inputs) -> np.ndarray:
    tokens = np.asarray(inputs["tokens"])
    args = prep_args(inputs)
    flags = (
        bool(np.any(args["b1"])),
        bool(np.any(args["b2"])),
        bool(np.any(args["gamma"] != 1.0)),
        bool(np.any(args["beta"])),
        bool(np.any(args["bout"])),
    )
    nc = _get_nc(flags)
    tok32 = np.ascontiguousarray(tokens.astype(np.int32))
    in_maps = [dict(args, tokens=tok32[c]) for c in range(NCORES)]
    res = run_bass_kernel_spmd(nc, in_maps, list(range(NCORES)))
    return np.stack([res.results[c]["out"] for c in range(NCORES)], axis=0)


if __name__ == "__main__":
    rng = np.random.default_rng(0)
    toy = {
        "tokens": rng.integers(0, V, size=(N, L)),
        "embed": rng.standard_normal((V, F)).astype(np.float32) * 0.02,
        "Wq": rng.standard_normal((NL, F, HQ)).astype(np.float32) * 0.02,
        "Wk": rng.standard_normal((NL, F, H * KD)).astype(np.float32) * 0.02,
        "Wv": rng.standard_normal((NL, F, H * KD)).astype(np.float32) * 0.02,
        "W1": rng.standard_normal((NL, HQ, F)).astype(np.float32) * 0.02,
        "b1": np.zeros((NL, F), np.float32),
        "W2": rng.standard_normal((NL, F, F)).astype(np.float32) * 0.02,
        "b2": np.zeros((NL, F), np.float32),
        "gamma": np.ones((NL, F), np.float32),
        "beta": np.zeros((NL, F), np.float32),
        "Wout": rng.standard_normal((F, V)).astype(np.float32) * 0.02,
        "bout": np.zeros((V,), np.float32),
    }
    o = kernel(**toy)
    print("out:", o.shape, o.dtype, float(np.abs(o).max()))


# revision 59
# speedup vs baseline: 1.6476x; 1.2937x over previous
"""Trainium2 Bass kernel for nn_AttentionModel (4-layer dense transformer).

Contract: kernel(**inputs) takes FULL unsharded inputs (as produced by
setup_inputs) and returns the FULL output [N, L, V] fp32.

Sharding: data-parallel over batch N=8 across the 8 NeuronCores — each core
runs the complete transformer for one batch element (identical NEFF, per-core
tokens). No collectives needed; the host stacks the per-core outputs.

Per-core dataflow (L=1024, F=512, H=8, KD=QD=64, NL=4, V=1024):
  - embedding: indirect-DMA gather of embed rows by token -> x0 natural [L, F]
  - activations kept in two layouts:
      natural [l(128-part) x F]  - for layernorm / residual / softmax scales
      T       [F(128-part) x L]  - as matmul operands (contraction on
      partitions); the T copy consumed by Q/K/V is fp8e4m3 (xT8), the one
      consumed by the unembed stays f32r. PE-transposes convert layouts.
  - per layer (emission interleaved for cross-engine overlap — see the
    schedule comment in the layer loop):
      kT = Wk^T x^T, vT = Wv^T x^T: fp8 DoubleRow matmuls (two 256-deep
           K-passes instead of four 128-deep f32r passes, 4x fewer PE
           cycles); psum pair-tile -> one f32r copy per output chunk
      q  = x Wq fp8 DoubleRow, stored fp16 as [j-chunk, head, 65] with a
           ones column so the attend matmul also produces softmax row-sums
      scores^T[j,i] = v k^T per head in f32r (K=64 matmuls on disjoint PE
           row-groups per head pair; causal tiles only)
      att_u = exp(scores^T - 5) in fp16: both heads of a pair share one
           2-bank psum tile so exp runs as a single wide ACT instruction;
           diagonal tiles triangle-zeroed in place with one gpsimd
           affine_select per pair (keep j<=i)
      x_new[i-block, pair] = att_u^T @ [q | 1] (fp16 matmuls, one psum bank
           per pair): col 64 of each head = softmax row-sum; strided
           reciprocal + 0-stride-broadcast multiply normalize on DVE
      x_newT via PE transposes (4 outputs share one psum bank -> one merged
           copy); MLP h1T = relu(W1^T x_newT + b1) (f32r, ACT relu+bias,
           one [P,2,512] psum pair per output chunk); h = h1T^T W2 (f32r,
           l-block pairs); y = LN(x + h) (bn_stats/bn_aggr on DVE, rstd on
           ACT, apply on gpsimd); yT via PE transposes -> xT8 (fp8) or xTr
           (f32r, last layer, feeds unembed)
  - unembed: logits = x4 Wout + bout in f32r, one [128, 1024] DMA per block.

Engine budget notes: gpsimd (Pool) cannot touch PSUM on TRN2, so all
psum->sbuf traffic is on DVE/ACT (split via TCOPY) and Pool takes the
SBUF-only work (affine_select, LN apply). The ACT table-set choice is pinned
(see _Bacc) so Exp/Ln/Relu/Copy share one loaded set - no per-layer ~2.7us
table swaps. fp8 is limited to the Q/K/V projections: k/v quantization noise
is washed out by the softmax ratio, q noise by attention averaging; scores,
attend, MLP and unembed keep their f32r/fp16 envelopes (measured end-to-end
rel err vs fp32 reference 5.5e-3, budget 2e-2; MLP in fp8 DR was measured
at 1.1e-2 with no speed gain, hence MLP_FP8=False).

Measured on hw (8-core SPMD, on-device 1001-iter loop differencing):
594,579 ns vs the 643,982 ns session baseline. Run-to-run variance on this
part is large (identical builds measured 594-757 us across sessions), so
knob decisions were made via same-process A/Bs only.
"""

import numpy as np

import concourse.bass as bass
import concourse.mybir as mybir
import concourse.tile as tile
from concourse import bacc
from concourse.bass_utils import run_bass_kernel_spmd
from concourse.masks import make_identity

# Model dims (hardcoded per the problem spec)
V, F, NL, H, KD, QD = 1024, 512, 4, 8, 64, 64
N, L = 8, 1024
HQ = H * QD  # 512
P = 128
FC = F // P      # 4 f-chunks
LB = L // P      # 8 l-blocks of 128
NCORES = 8

f32 = mybir.dt.float32
f32r = mybir.dt.float32r
f16 = mybir.dt.float16
f8 = mybir.dt.float8e4
i32 = mybir.dt.int32
AF = mybir.ActivationFunctionType
OP = mybir.AluOpType
DR = mybir.MatmulPerfMode.DoubleRow

_NC_CACHE: dict = {}
ABLATE = "none"  # perf-analysis knob: none|scores|attend|transposes
DR_MODE = "dr"  # q/k/v projection matmul mode: dr (fp8 DoubleRow, 256-deep
# K per pass) | fp8 (plain fp8, 128-deep chunks — isolates DoubleRow's real
# hw throughput from the fp8 layout changes)
MLP_FP8 = False  # MLP1/MLP2 in fp8 DoubleRow (x_newT/h1T stored fp8);
# False keeps the f32r MLP path (measured faster on hw in-process A/B and
# halves the end-to-end error: 5.5e-3 vs 1.1e-2)
LN_BATCH = False  # batch the LN ln/exp across the 8 l-chunks
TCOPY = "dve"  # engine for merged y/x0 transpose copies: dve|act|split
# (674us vs 683us for act in same-process hw A/B)
EXPP_BUFS = 22  # in-flight fp16 att PAIR tiles ([P,2,512]); the interleaved
# schedule keeps pair p's 12 tiles live while pair p+1's 12 are produced
PSUM_CFG = (3, 1, 1)  # bufs for (pp2, pa, pt). pp2 tiles are [P,2,512]
# (2 banks, shared by scores pairs / projection pairs / mlp / unembed); pa
# packs 2 attend accumulators of 130 f32 into one bank; pt packs 4 transpose
# outputs into one bank. Banks: 3*2 + 1 + 1 = 8.


class _Bacc(bacc.Bacc):
    """Bacc with activation-table-set selection pinned to
    natural_log_exp_and_others (contains Exp, Ln, Relu, Copy — everything this
    kernel uses) so the load-insertion pass emits one table load instead of
    thrashing between per-function sets (~2.7us per swap)."""

    def insert_act_table_loads(self):
        from concourse.hw_specs import get_activation_tables
        import concourse.mybir as _mb

        has_activation = any(
            isinstance(i, _mb.InstActivation)
            for b in self.main_func.blocks
            for i in b.instructions
        )
        if not has_activation:
            return
        keep = {AF.Exp, AF.Ln, AF.Relu, AF.Copy}
        chosen = "natural_log_exp_and_others"
        full = get_activation_tables(self.m.arch)
        assert keep <= full[chosen], (chosen, keep - full[chosen])
        tables = [
            (name, (fns if name == chosen else fns - keep))
            for name, fns in full.items()
        ]
        import bass_rust as _bass_rust
        _bass_rust.insert_act_table_loads(self, tables)


def _ln_apply(nc, y, b, mv8, rstd8, use_gamma, use_beta, gamma_b, beta_b):
    t = y[:, b, :]
    # DVE: same queue as bn_stats/bn_aggr, so the whole LN tail has a single
    # DVE->ACT->DVE cascade per layer instead of per-block engine hops
    nc.vector.tensor_scalar(
        t, t, mv8[:, b, 0:1], rstd8[:, b:b + 1],
        op0=OP.subtract, op1=OP.mult)
    if use_gamma:
        nc.vector.tensor_mul(t, t, gamma_b[:])
    if use_beta:
        nc.vector.tensor_add(t, t, beta_b[:])


def _r(ap):
    """View a DRAM fp32 AP as float32r for DMA into f32r tiles."""
    return ap.bitcast(f32r)


def _build(flags, repeat=1):
    use_b1, use_b2, use_gamma, use_beta, use_bout = flags
    nc = _Bacc("TRN2", target_bir_lowering=False, debug=False,
               num_devices=NCORES)

    tokens = nc.declare_dram_parameter("tokens", [L], i32, isOutput=False)
    embed = nc.declare_dram_parameter("embed", [V, F], f32, isOutput=False)
    Wq8 = nc.declare_dram_parameter("Wq8", [NL, F, HQ], f8, isOutput=False)
    Wk8 = nc.declare_dram_parameter("Wk8", [NL, F, H * KD], f8, isOutput=False)
    Wv8 = nc.declare_dram_parameter("Wv8", [NL, F, H * KD], f8, isOutput=False)
    if MLP_FP8:
        W18 = nc.declare_dram_parameter("W18", [NL, HQ, F], f8, isOutput=False)
        W28 = nc.declare_dram_parameter("W28", [NL, F, F], f8, isOutput=False)
    W1 = nc.declare_dram_parameter("W1", [NL, HQ, F], f32, isOutput=False)
    b1 = nc.declare_dram_parameter("b1", [NL, F], f32, isOutput=False)
    W2 = nc.declare_dram_parameter("W2", [NL, F, F], f32, isOutput=False)
    b2 = nc.declare_dram_parameter("b2", [NL, F], f32, isOutput=False)
    gamma = nc.declare_dram_parameter("gamma", [NL, F], f32, isOutput=False)
    beta = nc.declare_dram_parameter("beta", [NL, F], f32, isOutput=False)
    Wout = nc.declare_dram_parameter("Wout", [F, V], f32, isOutput=False)
    bout = nc.declare_dram_parameter("bout", [V], f32, isOutput=False)
    out = nc.declare_dram_parameter("out", [L, V], f32, isOutput=True)

    with tile.TileContext(nc) as tc:
        with (
            tc.tile_pool(name="bigT", bufs=3) as bigT,    # [P, FC, L] f32r
            tc.tile_pool(name="t8", bufs=4 if MLP_FP8 else 2) as t8p,       # [P, FC, L] fp8
            tc.tile_pool(name="nat", bufs=3) as natp,     # [P, LB, F] f32
            tc.tile_pool(name="qp", bufs=1) as qp,        # [P, LB, H, 65] f16
            tc.tile_pool(name="expp", bufs=EXPP_BUFS) as expp,  # [P, 2, 512] f16
            tc.tile_pool(name="wp", bufs=4) as wp,
            tc.tile_pool(name="cst", bufs=1) as cst,
            tc.tile_pool(name="sm", bufs=16) as sm,       # small per-partition scalars
            tc.tile_pool(name="op", bufs=2) as outp,      # [P, 1024] out staging
            tc.tile_pool(name="pp2", bufs=PSUM_CFG[0], space="PSUM") as pp2,
            tc.tile_pool(name="pa", bufs=PSUM_CFG[1], space="PSUM") as pa,
            tc.tile_pool(name="pt", bufs=PSUM_CFG[2], space="PSUM") as pt,
        ):
            # ---- constants ----
            ident = cst.tile([P, P], f32, tag="ident")
            make_identity(nc, ident[:])
            eps_t = cst.tile([P, 1], f32, tag="eps")
            nc.vector.memset(eps_t[:], 1e-5)
            neg5_t = cst.tile([P, 1], f32, tag="neg5")
            nc.vector.memset(neg5_t[:], -5.0)
            if use_b1:
                b1_sb = cst.tile([P, NL, FC], f32, tag="b1")
                nc.sync.dma_start(b1_sb[:], b1.rearrange("l (c p) -> p l c", p=P))
            if use_bout:
                bout_b = cst.tile([P, V], f32, tag="bout")
                bout_ap = bout[:]
                nc.sync.dma_start(
                    bout_b[:],
                    bass.AP(tensor=bout_ap.tensor, offset=bout_ap.offset,
                            ap=[[0, P]] + bout_ap.ap),
                )

            def bcast_row(dram_row_ap, tag):
                t = cst.tile([P, F], f32, tag=tag)
                nc.sync.dma_start(
                    t[:],
                    bass.AP(tensor=dram_row_ap.tensor, offset=dram_row_ap.offset,
                            ap=[[0, P]] + dram_row_ap.ap),
                )
                return t

            import contextlib
            _loop = (tc.For_i(0, repeat, 1) if repeat > 1
                     else contextlib.nullcontext())
            with _loop:
                # ---- embedding gather ----
                tok_sb = cst.tile([P, LB], i32, tag="tok")
                nc.sync.dma_start(tok_sb[:], tokens.rearrange("(b p) -> p b", p=P))
                x_nat = natp.tile([P, LB, F], f32, tag="nat")
                if ABLATE == "embed":
                    nc.gpsimd.memset(x_nat[:], 0.02)
                else:
                    for b in range(LB):
                        nc.gpsimd.indirect_dma_start(
                            out=x_nat[:, b, :], out_offset=None,
                            in_=embed[:],
                            in_offset=bass.IndirectOffsetOnAxis(ap=tok_sb[:, b:b + 1], axis=0),
                        )

                def tcopy(i, dst, src):
                    """Merged psum->sbuf copy; engine per TCOPY."""
                    if TCOPY == "dve" or (TCOPY == "split" and i % 2 == 0):
                        nc.vector.tensor_copy(dst, src)
                    else:
                        nc.scalar.copy(dst, src)

                def transpose_blocks(src_nat, dst_T, b0, nb):
                    """Transpose l-blocks [b0, b0+nb) of natural [P, LB, F]
                    f32 into T layout [P, FC, L] (dtype cast per dst tile:
                    f32r or fp8). 4 transpose outputs share one psum bank and
                    drain with a single merged copy."""
                    if ABLATE == "transposes":
                        nc.gpsimd.memset(dst_T[:, :, b0 * P:(b0 + nb) * P], 0.1)
                        return
                    for b in range(b0, b0 + nb):
                        pt_ps = pt.tile([P, FC, P], f32, tag="pt")
                        for c in range(FC):
                            nc.tensor.transpose(
                                pt_ps[:, c, :],
                                src_nat[:, b, c * P:(c + 1) * P], ident[:])
                        tcopy(b, dst_T[:, :, b * P:(b + 1) * P], pt_ps[:])

                xT8 = t8p.tile([P, FC, L], f8, tag="t8")
                transpose_blocks(x_nat, xT8, 0, LB)

                # ---- layers ----
                for li in range(NL):
                    last = li == NL - 1
                    wq8_t = wp.tile([P, FC, HQ], f8, tag="w8", bufs=5 if MLP_FP8 else 3)
                    wk8_t = wp.tile([P, FC, HQ], f8, tag="w8", bufs=5 if MLP_FP8 else 3)
                    wv8_t = wp.tile([P, FC, HQ], f8, tag="w8", bufs=5 if MLP_FP8 else 3)
                    if ABLATE != "wdma":
                        nc.sync.dma_start(wq8_t[:], Wq8[li].rearrange("(c p) o -> p c o", p=P))
                        nc.sync.dma_start(wk8_t[:], Wk8[li].rearrange("(c p) o -> p c o", p=P))
                        nc.sync.dma_start(wv8_t[:], Wv8[li].rearrange("(c p) o -> p c o", p=P))
                    if MLP_FP8:
                        w1_t = wp.tile([P, FC, F], f8, tag="w8", bufs=5 if MLP_FP8 else 3)
                        w2_t = wp.tile([P, FC, F], f8, tag="w8", bufs=5 if MLP_FP8 else 3)
                        nc.sync.dma_start(
                            w1_t[:], W18[li].rearrange("(c p) o -> p c o", p=P))
                        nc.sync.dma_start(
                            w2_t[:], W28[li].rearrange("(c p) o -> p c o", p=P))
                    else:
                        w1_t = wp.tile([P, FC, F], f32r, tag="w", bufs=4)
                        w2_t = wp.tile([P, FC, F], f32r, tag="w", bufs=4)
                        if ABLATE != "wdma":
                            nc.sync.dma_start(
                                w1_t[:], _r(W1[li].rearrange("(c p) o -> p c o", p=P)))
                            nc.sync.dma_start(
                                w2_t[:], _r(W2[li].rearrange("(c p) o -> p c o", p=P)))

                    # kT, vT chunk oc: fp8 DoubleRow (256-deep K per pass),
                    # one [P,2,512] psum pair -> one 1024-wide f32r copy
                    kT = bigT.tile([P, FC, L], f32r, tag="bigT")
                    vT = bigT.tile([P, FC, L], f32r, tag="bigT")

                    def emit_kv(oc):
                        for wi, (w8_t, oT) in enumerate(((wk8_t, kT), (wv8_t, vT))):
                            ps = pp2.tile([P, 2, 512], f32, tag="pp2")
                            if DR_MODE == "dr":
                                # fc2 outer / lc inner: consecutive matmuls
                                # share the stationary -> one ldweights per
                                # weight block
                                for fc2 in range(0, FC, 2):
                                    for lc in range(2):
                                        nc.tensor.matmul(
                                            ps[:, lc, :],
                                            w8_t[:, fc2:fc2 + 2, oc * P:(oc + 1) * P],
                                            xT8[:, fc2:fc2 + 2, lc * 512:(lc + 1) * 512],
                                            start=(fc2 == 0), stop=(fc2 == FC - 2),
                                            perf_mode=DR)
                            else:
                                for fc in range(FC):
                                    for lc in range(2):
                                        nc.tensor.matmul(
                                            ps[:, lc, :],
                                            w8_t[:, fc, oc * P:(oc + 1) * P],
                                            xT8[:, fc, lc * 512:(lc + 1) * 512],
                                            start=(fc == 0), stop=(fc == FC - 1))
                            # chunk 0 drains on ACT: it gates the layer's
                            # first scores while DVE still holds the previous
                            # layer's transpose-copy backlog
                            eng = (nc.scalar.copy if oc == 0
                                   else nc.vector.tensor_copy)
                            eng(oT[:, oc, :],
                                ps[:].rearrange("p a b -> p (a b)"))

                    # q natural (fp16 for the attend matmul), [P(j), jc, head, 65]
                    # with a trailing ones column so attend also yields row-sums
                    q_sb = qp.tile([P, LB, H, 65], f16, tag="q")
                    nc.vector.memset(q_sb[:, :, :, 64:65], 1.0)

                    def emit_q2(bp):
                        ps = pp2.tile([P, 2, 512], f32, tag="pp2")
                        for i2 in range(2):
                            b = 2 * bp + i2
                            if DR_MODE == "dr":
                                for fc2 in range(0, FC, 2):
                                    nc.tensor.matmul(
                                        ps[:, i2, :],
                                        xT8[:, fc2:fc2 + 2, b * P:(b + 1) * P],
                                        wq8_t[:, fc2:fc2 + 2, :],
                                        start=(fc2 == 0), stop=(fc2 == FC - 2),
                                        perf_mode=DR)
                            else:
                                for fc in range(FC):
                                    nc.tensor.matmul(
                                        ps[:, i2, :],
                                        xT8[:, fc, b * P:(b + 1) * P],
                                        wq8_t[:, fc, :],
                                        start=(fc == 0), stop=(fc == FC - 1))
                        eng = nc.vector.tensor_copy if bp % 2 else nc.scalar.copy
                        eng(q_sb[:, 2 * bp:2 * bp + 2, :, 0:64],
                            ps[:].rearrange("p b (h d) -> p b h d", h=H))

                    x_new = natp.tile([P, LB, F], f32, tag="nat")
                    if MLP_FP8:
                        x_newT = t8p.tile([P, FC, L], f8, tag="t8")
                    else:
                        x_newT = bigT.tile([P, FC, L], f32r, tag="bigT")
                    exp_store: dict = {}

                    def emit_xnewT(p, half):
                        # transposes of x_new chunk p (head pair p's columns)
                        # for the 4 l-blocks finished by attend(p, half).
                        # Pair 3's copies gate MLP1 and run when the exps are
                        # done, so they drain on the then-idle ACT.
                        pt_ps = pt.tile([P, 4, P], f32, tag="pt")
                        for i, b in enumerate(range(4 * half, 4 * half + 4)):
                            nc.tensor.transpose(
                                pt_ps[:, i, :],
                                x_new[:, b, p * P:(p + 1) * P], ident[:])
                        eng = nc.scalar.copy if p == 3 else nc.vector.tensor_copy
                        eng(x_newT[:, p, 4 * half * P:(4 * half + 4) * P],
                            pt_ps[:])

                    def emit_scores(hpair, c):
                        heads = (2 * hpair, 2 * hpair + 1)
                        hc = hpair
                        tiles = {}
                        for jc in range(4 * c + 4):
                            d = jc - 4 * c
                            n0 = 0 if d < 0 else min(P * d, 256)
                            e0 = 0 if d < 0 else P * d
                            # both heads of the pair in one 2-bank psum tile
                            # so exp / affine_select run as single wide
                            # instructions (halves the per-instr ACT access
                            # latency spend)
                            ps = pp2.tile([P, 2, 512], f32, tag="pp2")
                            if ABLATE != "scores":
                                for hi, h in enumerate(heads):
                                    hb = 64 * (h % 2)
                                    nc.tensor.matmul(
                                        ps[:, hi, n0:512],
                                        vT[hb:hb + KD, hc, jc * P:(jc + 1) * P],
                                        kT[hb:hb + KD, hc, c * 512 + n0:(c + 1) * 512],
                                        start=True, stop=True)
                            et = expp.tile([P, 2, 512], f16, tag="exp")
                            # bias=-5: softmax is shift-invariant (both the
                            # attend numerator and the ones-column row-sum
                            # scale by e^-5), keeps exp within fp16 range
                            if ABLATE != "scores":
                                nc.scalar.activation(
                                    et[:, :, e0:512], ps[:, :, e0:512], AF.Exp,
                                    bias=neg5_t[:])
                                if d >= 0:
                                    # zero att where j > i (in-place triangle
                                    # select over both heads at once)
                                    nc.gpsimd.affine_select(
                                        out=et[:, :, e0:e0 + P],
                                        in_=et[:, :, e0:e0 + P],
                                        compare_op=OP.is_ge,
                                        fill=0.0, base=0,
                                        pattern=[[0, 2], [1, P]],
                                        channel_multiplier=-1)
                            else:
                                nc.gpsimd.memset(et[:, :, e0:512], 0.5)
                            tiles[jc] = et
                        exp_store[(hpair, c)] = tiles

                    def emit_attend(hpair, c):
                        heads = (2 * hpair, 2 * hpair + 1)
                        tiles = exp_store.pop((hpair, c))
                        # two b-slots share one psum bank (2 x 130 f32);
                        # normalization is batched per b-pair: one strided
                        # reciprocal + one broadcast multiply for both slots
                        pa_t = pa.tile([P, 2, 130], f32, tag="pa")
                        for b0 in range(4 * c, 4 * c + 4, 2):
                            if ABLATE == "attend":
                                for h in heads:
                                    nc.gpsimd.memset(
                                        x_new[:, b0:b0 + 2,
                                              h * 64:(h + 1) * 64], 0.1)
                                continue
                            for s, b in enumerate((b0, b0 + 1)):
                                lc0 = (b - 4 * c) * P
                                # both heads of the pair accumulate into one
                                # psum bank: head h' at cols [65*h', 65*h'+65)
                                for hi, h in enumerate(heads):
                                    for jc in range(b + 1):
                                        nc.tensor.matmul(
                                            pa_t[:, s, 65 * hi:65 * hi + 65],
                                            tiles[jc][:, hi, lc0:lc0 + P],
                                            q_sb[:, jc, h, :],
                                            start=(jc == 0), stop=(jc == b))
                            pa4 = pa_t[:].rearrange("p s (h x) -> p s h x", h=2)
                            rc = sm.tile([P, 2, 2], f32, tag="rc")
                            nc.vector.reciprocal(rc[:], pa4[:, :, :, 64])
                            # x_new[:, b0:b0+2, pair] = att_u @ q * recip
                            # (recip broadcast 64-wide per head, 0-stride)
                            xdst = x_new[:, b0:b0 + 2,
                                         hpair * P:(hpair + 1) * P].rearrange(
                                "p b (h x) -> p b h x", h=2)
                            nc.vector.tensor_tensor(
                                xdst, pa4[:, :, :, 0:64],
                                rc[:, :, :, None].to_broadcast((P, 2, 2, 64)),
                                OP.mult)

                    # schedule: kv chunk p -> attends of pair p-1 (+ their
                    # transposes) -> scores of pair p. Scores sit last in each
                    # iteration because the pp2 ring throttles them to ACT's
                    # exp pace — everything PE-independent is emitted first.
                    for p in range(H // 2):
                        emit_kv(p)
                        if p == 0:
                            emit_scores(0, 0)
                            emit_scores(0, 1)
                            emit_q2(0)
                            emit_q2(1)
                        else:
                            if p == 1:
                                emit_q2(2)
                                emit_q2(3)
                            emit_attend(p - 1, 0)
                            emit_xnewT(p - 1, 0)
                            emit_attend(p - 1, 1)
                            emit_xnewT(p - 1, 1)
                            emit_scores(p, 0)
                            emit_scores(p, 1)
                    emit_attend(3, 0)
                    emit_xnewT(3, 0)
                    emit_attend(3, 1)
                    emit_xnewT(3, 1)

                    # MLP1: h1T = relu(W1^T x_newT + b1), one [P,2,512] psum
                    # pair + one wide ACT relu per out chunk; fc outer / lc
                    # inner so consecutive matmuls share the stationary
                    if MLP_FP8:
                        h1T = t8p.tile([P, FC, L], f8, tag="t8")
                    else:
                        h1T = bigT.tile([P, FC, L], f32r, tag="bigT")
                    for oc in range(FC):
                        ps = pp2.tile([P, 2, 512], f32, tag="pp2")
                        if MLP_FP8:
                            for fc2 in range(0, FC, 2):
                                for lc in range(2):
                                    nc.tensor.matmul(
                                        ps[:, lc, :],
                                        w1_t[:, fc2:fc2 + 2, oc * P:(oc + 1) * P],
                                        x_newT[:, fc2:fc2 + 2,
                                               lc * 512:(lc + 1) * 512],
                                        start=(fc2 == 0), stop=(fc2 == FC - 2),
                                        perf_mode=DR)
                        else:
                            for fc in range(FC):
                                for lc in range(2):
                                    nc.tensor.matmul(
                                        ps[:, lc, :],
                                        w1_t[:, fc, oc * P:(oc + 1) * P],
                                        x_newT[:, fc, lc * 512:(lc + 1) * 512],
                                        start=(fc == 0), stop=(fc == FC - 1))
                        bias = b1_sb[:, li, oc:oc + 1] if use_b1 else 0.0
                        nc.scalar.activation(
                            h1T[:, oc, :], ps[:].rearrange("p a b -> p (a b)"),
                            AF.Relu, bias=bias)

                    # MLP2 + residual + LN -> y, l-blocks in pairs, with the
                    # y transposes (and, last layer, the unembed) interleaved
                    # per pair so the layer-boundary chain stays pipelined
                    if use_b2:
                        b2_b = bcast_row(b2[li], f"b2_{li}")
                    if use_gamma:
                        gamma_b = bcast_row(gamma[li], f"g_{li}")
                    if use_beta:
                        beta_b = bcast_row(beta[li], f"be_{li}")
                    if last:
                        wo = []
                        for vc in range(2):
                            wt = wp.tile([P, FC, 512], f32r, tag="w", bufs=4)
                            nc.sync.dma_start(
                                wt[:],
                                _r(Wout[:, vc * 512:(vc + 1) * 512]
                                   .rearrange("(c p) o -> p c o", p=P)))
                            wo.append(wt)
                        xT_next = bigT.tile([P, FC, L], f32r, tag="bigT")
                    else:
                        xT_next = t8p.tile([P, FC, L], f8, tag="t8")
                    y = natp.tile([P, LB, F], f32, tag="nat")
                    mv8 = sm.tile([P, LB, 2], f32, tag="mv8")
                    rstd8 = sm.tile([P, LB], f32, tag="rs8")

                    def emit_unembed(b):
                        # fc outer / vc inner: consecutive matmuls share the
                        # stationary xT block -> one ldweights per fc
                        ps = pp2.tile([P, 2, 512], f32, tag="pp2")
                        for fc in range(FC):
                            for vc in range(2):
                                nc.tensor.matmul(
                                    ps[:, vc, :],
                                    xT_next[:, fc, b * P:(b + 1) * P],
                                    wo[vc][:, fc, :],
                                    start=(fc == 0), stop=(fc == FC - 1))
                        if ABLATE == "outdma":
                            return
                        ot = outp.tile([P, V], f32, tag="o")
                        psf = ps[:].rearrange("p a b -> p (a b)")
                        if use_bout:
                            nc.vector.tensor_add(ot[:], psf, bout_b[:])
                        else:
                            eng = (nc.vector.tensor_copy if b % 2
                                   else nc.scalar.copy)
                            eng(ot[:], psf)
                        nc.sync.dma_start(out[b * P:(b + 1) * P, :], ot[:])

                    for bp in range(LB // 2):
                        ps = pp2.tile([P, 2, 512], f32, tag="pp2")
                        for i2 in range(2):
                            b = 2 * bp + i2
                            if MLP_FP8:
                                for fc2 in range(0, FC, 2):
                                    nc.tensor.matmul(
                                        ps[:, i2, :],
                                        h1T[:, fc2:fc2 + 2, b * P:(b + 1) * P],
                                        w2_t[:, fc2:fc2 + 2, :],
                                        start=(fc2 == 0), stop=(fc2 == FC - 2),
                                        perf_mode=DR)
                            else:
                                for fc in range(FC):
                                    nc.tensor.matmul(
                                        ps[:, i2, :],
                                        h1T[:, fc, b * P:(b + 1) * P],
                                        w2_t[:, fc, :],
                                        start=(fc == 0), stop=(fc == FC - 1))
                        t2 = y[:, 2 * bp:2 * bp + 2, :]
                        nc.vector.tensor_add(t2, ps[:], x_nat[:, 2 * bp:2 * bp + 2, :])
                        if use_b2:
                            nc.vector.tensor_add(
                                t2, t2,
                                b2_b[:, None, :].to_broadcast((P, 2, F)))
                        for i2 in range(2):
                            b = 2 * bp + i2
                            if ABLATE == "ln":
                                continue
                            st = sm.tile([P, 6], f32, tag="st")
                            nc.vector.bn_stats(st[:], y[:, b, :])
                            nc.vector.bn_aggr(mv8[:, b, :], st[:])
                    # batched rstd = exp(-0.5*ln(var+eps)) for all 8 blocks:
                    # two [P,8] ACT ops instead of 16 tiny ones — the hw
                    # charges ~us-scale latency per cross-engine cascade, so
                    # one cascade per layer beats a pipelined-looking
                    # per-block chain by ~50us/layer (measured via ablation)
                    if ABLATE != "ln":
                        nc.scalar.activation(rstd8[:], mv8[:, :, 1], AF.Ln,
                                             bias=eps_t[:])
                        nc.scalar.activation(rstd8[:], rstd8[:], AF.Exp,
                                             scale=-0.5)
                    for bp in range(LB // 2):
                        if ABLATE != "ln":
                            for i2 in range(2):
                                b = 2 * bp + i2
                                _ln_apply(nc, y, b, mv8, rstd8, use_gamma,
                                          use_beta,
                                          gamma_b if use_gamma else None,
                                          beta_b if use_beta else None)
                        transpose_blocks(y, xT_next, 2 * bp, 2)
                        if last:
                            emit_unembed(2 * bp)
                            emit_unembed(2 * bp + 1)

                    x_nat = y
                    xT8 = xT_next
    nc.compile()
    return nc


def _get_nc(flags, repeat=1):
    key = (flags, repeat, ABLATE, LN_BATCH, PSUM_CFG, TCOPY, EXPP_BUFS,
           DR_MODE)
    if key not in _NC_CACHE:
        _NC_CACHE[key] = _build(flags, repeat)
    return _NC_CACHE[key]


def make_runner(flags, in_maps, repeat=1):
    """Build a reusable jitted SPMD runner with device-resident inputs.

    Returns (run, split_outputs) where run() executes the kernel once on all
    8 cores and blocks; used by test.py for timing without per-call host->device
    input transfer.
    """
    import jax
    from jax.sharding import Mesh, PartitionSpec, NamedSharding
    from concourse import bass2jax, mybir as _mybir

    bass2jax.install_neuronx_cc_hook()
    nc = _get_nc(flags, repeat)
    partition_name = (nc.partition_id_tensor.name if nc.partition_id_tensor
                      else None)
    in_names, out_names, out_avals, zero_outs = [], [], [], []
    for alloc in nc.m.functions[0].allocations:
        if not isinstance(alloc, _mybir.MemoryLocationSet):
            continue
        name = alloc.memorylocations[0].name
        if alloc.kind == "ExternalInput":
            if name != partition_name:
                in_names.append(name)
        elif alloc.kind == "ExternalOutput":
            shape = tuple(alloc.tensor_shape)
            dtype = _mybir.dt.np(alloc.dtype)
            out_names.append(name)
            out_avals.append(jax.core.ShapedArray(shape, dtype))
            zero_outs.append(np.zeros(shape, dtype))
    n_params = len(in_names)
    n_outs = len(out_avals)
    all_names = in_names + out_names + ([partition_name] if partition_name else [])

    def _body(*args):
        operands = list(args)
        if partition_name is not None:
            operands.append(bass2jax.partition_id_tensor())
        outs = bass2jax._bass_exec_p.bind(
            *operands,
            out_avals=tuple(out_avals),
            in_names=tuple(all_names),
            out_names=tuple(out_names),
            lowering_input_output_aliases=(),
            sim_require_finite=True,
            sim_require_nnan=True,
            nc=nc,
        )
        return tuple(outs)

    from jax.experimental.shard_map import shard_map
    devices = jax.devices()[:NCORES]
    mesh = Mesh(np.asarray(devices), ("core",))
    in_specs = (PartitionSpec("core"),) * (n_params + n_outs)
    out_specs = (PartitionSpec("core"),) * n_outs
    sharded = jax.jit(
        shard_map(_body, mesh=mesh, in_specs=in_specs, out_specs=out_specs,
                  check_rep=False),
        keep_unused=True,
    )
    concat_in = [
        np.concatenate([np.asarray(in_maps[c][nm])[None] for c in range(NCORES)],
                       axis=0).reshape(NCORES * np.asarray(in_maps[0][nm]).shape[0],
                                       *np.asarray(in_maps[0][nm]).shape[1:])
        for nm in in_names
    ]
    sh = NamedSharding(mesh, PartitionSpec("core"))
    dev_in = [jax.device_put(x, sh) for x in concat_in]
    dev_zeros = [
        jax.device_put(np.zeros((NCORES * z.shape[0], *z.shape[1:]), z.dtype), sh)
        for z in zero_outs
    ]

    def run():
        outs = sharded(*dev_in, *dev_zeros)
        jax.block_until_ready(outs)
        return outs

    def split(outs):
        return [
            {nm: np.asarray(outs[i]).reshape(NCORES, *out_avals[i].shape)[c]
             for i, nm in enumerate(out_names)}
            for c in range(NCORES)
        ]

    return run, split


def prep_args(inputs):
    """Host-side arg prep shared by kernel() and test.py: fp32 copies of the
    fp32 params, fp8e4m3 casts of Wq/Wk/Wv (consumed by the DoubleRow
    projection matmuls)."""
    import ml_dtypes
    args = {k: np.ascontiguousarray(np.asarray(v), dtype=np.float32)
            for k, v in inputs.items() if k not in ("tokens", "Wq", "Wk", "Wv")}
    for k in ("Wq", "Wk", "Wv"):
        args[k + "8"] = np.ascontiguousarray(
            np.asarray(inputs[k], dtype=np.float32).astype(ml_dtypes.float8_e4m3))
    if MLP_FP8:
        for k in ("W1", "W2"):
            args[k + "8"] = np.ascontiguousarray(
                args[k].astype(ml_dtypes.float8_e4m3))
    return args


def kernel(**inputs) -> np.ndarray:
    tokens = np.asarray(inputs["tokens"])
    args = prep_args(inputs)
    flags = (
        bool(np.any(args["b1"])),
        bool(np.any(args["b2"])),
        bool(np.any(args["gamma"] != 1.0)),
        bool(np.any(args["beta"])),
        bool(np.any(args["bout"])),
    )
    nc = _get_nc(flags)
    tok32 = np.ascontiguousarray(tokens.astype(np.int32))
    in_maps = [dict(args, tokens=tok32[c]) for c in range(NCORES)]
    res = run_bass_kernel_spmd(nc, in_maps, list(range(NCORES)))
    return np.stack([res.results[c]["out"] for c in range(NCORES)], axis=0)


if __name__ == "__main__":
    rng = np.random.default_rng(0)
    toy = {
        "tokens": rng.integers(0, V, size=(N, L)),
        "embed": rng.standard_normal((V, F)).astype(np.float32) * 0.02,
        "Wq": rng.standard_normal((NL, F, HQ)).astype(np.float32) * 0.02,
        "Wk": rng.standard_normal((NL, F, H * KD)).astype(np.float32) * 0.02,
        "Wv": rng.standard_normal((NL, F, H * KD)).astype(np.float32) * 0.02,
        "W1": rng.standard_normal((NL, HQ, F)).astype(np.float32) * 0.02,
        "b1": np.zeros((NL, F), np.float32),
        "W2": rng.standard_normal((NL, F, F)).astype(np.float32) * 0.02,
        "b2": np.zeros((NL, F), np.float32),
        "gamma": np.ones((NL, F), np.float32),
        "beta": np.zeros((NL, F), np.float32),
        "Wout": rng.standard_normal((F, V)).astype(np.float32) * 0.02,
        "bout": np.zeros((V,), np.float32),
    }
    o = kernel(**toy)
    print("out:", o.shape, o.dtype, float(np.abs(o).max()))


# revision 61
# speedup vs baseline: 1.8809x; 1.1416x over previous
"""Trainium2 Bass kernel for nn_AttentionModel (4-layer dense transformer).

Contract: kernel(**inputs) takes FULL unsharded inputs (as produced by
setup_inputs) and returns the FULL output [N, L, V] fp32.

Sharding: data-parallel over batch N=8 across the 8 NeuronCores — each core
runs the complete transformer for one batch element (identical NEFF, per-core
tokens). No collectives needed; the host stacks the per-core outputs.

Per-core dataflow (L=1024, F=512, H=8, KD=QD=64, NL=4, V=1024):
  - embedding: indirect-DMA gather of embed rows by token -> x0 natural [L, F]
  - activations kept in two layouts:
      natural [l(128-part) x F]  - for layernorm / residual / softmax scales
      T       [F(128-part) x L]  - as matmul operands (contraction on
      partitions); the T copy consumed by Q/K/V is fp8e4m3 (xT8), the one
      consumed by the unembed stays f32r. PE-transposes convert layouts.
  - per layer (emission interleaved for cross-engine overlap — see the
    schedule comment in the layer loop):
      kT = Wk^T x^T, vT = Wv^T x^T: fp8 DoubleRow matmuls (two 256-deep
           K-passes instead of four 128-deep f32r passes, 4x fewer PE
           cycles); psum pair-tile -> one f32r copy per output chunk
      q  = x Wq fp8 DoubleRow, stored fp16 as [j-chunk, head, 65] with a
           ones column so the attend matmul also produces softmax row-sums
      scores^T[j,i] = v k^T per head in f32r (K=64 matmuls on disjoint PE
           row-groups per head pair; causal tiles only)
      att_u = exp(scores^T - 5) in fp16: both heads of a pair share one
           2-bank psum tile so exp runs as a single wide ACT instruction;
           diagonal tiles triangle-zeroed in place with one gpsimd
           affine_select per pair (keep j<=i)
      x_new[i-block, pair] = att_u^T @ [q | 1] (fp16 matmuls, one psum bank
           per pair): col 64 of each head = softmax row-sum; strided
           reciprocal + 0-stride-broadcast multiply normalize on DVE
      x_newT via PE transposes (4 outputs share one psum bank -> one merged
           copy); MLP h1T = relu(W1^T x_newT + b1) (f32r, ACT relu+bias,
           one [P,2,512] psum pair per output chunk); h = h1T^T W2 (f32r,
           l-block pairs); y = LN(x + h) (bn_stats/bn_aggr on DVE, rstd on
           ACT, apply on gpsimd); yT via PE transposes -> xT8 (fp8) or xTr
           (f32r, last layer, feeds unembed)
  - unembed: logits = x4 Wout + bout in f32r, one [128, 1024] DMA per block.

Engine budget notes: gpsimd (Pool) cannot touch PSUM on TRN2, so all
psum->sbuf traffic is on DVE/ACT (split via TCOPY) and Pool takes the
SBUF-only work (affine_select, LN apply). The ACT table-set choice is pinned
(see _Bacc) so Exp/Ln/Relu/Copy share one loaded set - no per-layer ~2.7us
table swaps. fp8 is limited to the Q/K/V projections: k/v quantization noise
is washed out by the softmax ratio, q noise by attention averaging; scores,
attend, MLP and unembed keep their f32r/fp16 envelopes (measured end-to-end
rel err vs fp32 reference 5.5e-3, budget 2e-2; MLP in fp8 DR was measured
at 1.1e-2 with no speed gain, hence MLP_FP8=False).

Measured on hw (8-core SPMD, on-device 1001-iter loop differencing):
594,579 ns vs the 643,982 ns session baseline. Run-to-run variance on this
part is large (identical builds measured 594-757 us across sessions), so
knob decisions were made via same-process A/Bs only.
"""

import numpy as np

import concourse.bass as bass
import concourse.mybir as mybir
import concourse.tile as tile
from concourse import bacc
from concourse.bass_utils import run_bass_kernel_spmd
from concourse.masks import make_identity, make_upper_triangular

# Model dims (hardcoded per the problem spec)
V, F, NL, H, KD, QD = 1024, 512, 4, 8, 64, 64
N, L = 8, 1024
HQ = H * QD  # 512
P = 128
FC = F // P      # 4 f-chunks
LB = L // P      # 8 l-blocks of 128
NCORES = 8

f32 = mybir.dt.float32
f32r = mybir.dt.float32r
f16 = mybir.dt.float16
f8 = mybir.dt.float8e4
i32 = mybir.dt.int32
AF = mybir.ActivationFunctionType
OP = mybir.AluOpType
DR = mybir.MatmulPerfMode.DoubleRow

_NC_CACHE: dict = {}
ABLATE = "none"  # perf-analysis knob: none|scores|attend|transposes
DR_MODE = "dr"  # q/k/v projection matmul mode: dr (fp8 DoubleRow, 256-deep
# K per pass) | fp8 (plain fp8, 128-deep chunks — isolates DoubleRow's real
# hw throughput from the fp8 layout changes)
MLP_FP8 = False  # MLP1/MLP2 in fp8 DoubleRow (x_newT/h1T stored fp8);
# False keeps the f32r MLP path (measured faster on hw in-process A/B and
# halves the end-to-end error: 5.5e-3 vs 1.1e-2)
LN_BATCH = False  # batch the LN ln/exp across the 8 l-chunks
TCOPY = "dve"  # engine for merged y/x0 transpose copies: dve|act|split
# (674us vs 683us for act in same-process hw A/B)
TRIMASK = "pool"  # causal triangle zeroing of diagonal att tiles:
# dve (tensor_tensor multiply with a precomputed f16 mask, 4x mode, keeps
# the score->exp->attend cascade off the Pool engine) | pool (gpsimd
# affine_select, one fewer DVE op but an extra engine hop per diag tile)
EXPP_BUFS = 22  # in-flight fp16 att PAIR tiles ([P,2,512]); the interleaved
# schedule keeps pair p's 12 tiles live while pair p+1's 12 are produced
PSUM_CFG = (3, 1, 1)  # bufs for (pp2, pa, pt). pp2 tiles are [P,2,512]
# (2 banks, shared by scores pairs / projection pairs / mlp / unembed); pa
# packs 2 attend accumulators of 130 f32 into one bank; pt packs 4 transpose
# outputs into one bank. Banks: 3*2 + 1 + 1 = 8.


class _Bacc(bacc.Bacc):
    """Bacc with activation-table-set selection pinned to
    natural_log_exp_and_others (contains Exp, Ln, Relu, Copy — everything this
    kernel uses) so the load-insertion pass emits one table load instead of
    thrashing between per-function sets (~2.7us per swap)."""

    def insert_act_table_loads(self):
        from concourse.hw_specs import get_activation_tables
        import concourse.mybir as _mb

        has_activation = any(
            isinstance(i, _mb.InstActivation)
            for b in self.main_func.blocks
            for i in b.instructions
        )
        if not has_activation:
            return
        keep = {AF.Exp, AF.Ln, AF.Relu, AF.Copy}
        chosen = "natural_log_exp_and_others"
        full = get_activation_tables(self.m.arch)
        assert keep <= full[chosen], (chosen, keep - full[chosen])
        tables = [
            (name, (fns if name == chosen else fns - keep))
            for name, fns in full.items()
        ]
        import bass_rust as _bass_rust
        _bass_rust.insert_act_table_loads(self, tables)


def _ln_apply(nc, y, b, mv8, rstd8, use_gamma, use_beta, gamma_b, beta_b):
    t = y[:, b, :]
    # DVE: same queue as bn_stats/bn_aggr, so the whole LN tail has a single
    # DVE->ACT->DVE cascade per layer instead of per-block engine hops
    nc.vector.tensor_scalar(
        t, t, mv8[:, b, 0:1], rstd8[:, b:b + 1],
        op0=OP.subtract, op1=OP.mult)
    if use_gamma:
        nc.vector.tensor_mul(t, t, gamma_b[:])
    if use_beta:
        nc.vector.tensor_add(t, t, beta_b[:])


def _r(ap):
    """View a DRAM fp32 AP as float32r for DMA into f32r tiles."""
    return ap.bitcast(f32r)


def _build(flags, repeat=1):
    use_b1, use_b2, use_gamma, use_beta, use_bout = flags
    nc = _Bacc("TRN2", target_bir_lowering=False, debug=False,
               num_devices=NCORES)

    tokens = nc.declare_dram_parameter("tokens", [L], i32, isOutput=False)
    embed = nc.declare_dram_parameter("embed", [V, F], f32, isOutput=False)
    Wq8 = nc.declare_dram_parameter("Wq8", [NL, F, HQ], f8, isOutput=False)
    Wk8 = nc.declare_dram_parameter("Wk8", [NL, F, H * KD], f8, isOutput=False)
    Wv8 = nc.declare_dram_parameter("Wv8", [NL, F, H * KD], f8, isOutput=False)
    if MLP_FP8:
        W18 = nc.declare_dram_parameter("W18", [NL, HQ, F], f8, isOutput=False)
        W28 = nc.declare_dram_parameter("W28", [NL, F, F], f8, isOutput=False)
    W1 = nc.declare_dram_parameter("W1", [NL, HQ, F], f32, isOutput=False)
    b1 = nc.declare_dram_parameter("b1", [NL, F], f32, isOutput=False)
    W2 = nc.declare_dram_parameter("W2", [NL, F, F], f32, isOutput=False)
    b2 = nc.declare_dram_parameter("b2", [NL, F], f32, isOutput=False)
    gamma = nc.declare_dram_parameter("gamma", [NL, F], f32, isOutput=False)
    beta = nc.declare_dram_parameter("beta", [NL, F], f32, isOutput=False)
    Wout = nc.declare_dram_parameter("Wout", [F, V], f32, isOutput=False)
    bout = nc.declare_dram_parameter("bout", [V], f32, isOutput=False)
    out = nc.declare_dram_parameter("out", [L, V], f32, isOutput=True)

    with tile.TileContext(nc) as tc:
        with (
            tc.tile_pool(name="bigT", bufs=3) as bigT,    # [P, FC, L] f32r
            tc.tile_pool(name="t8", bufs=4 if MLP_FP8 else 2) as t8p,       # [P, FC, L] fp8
            tc.tile_pool(name="nat", bufs=3) as natp,     # [P, LB, F] f32
            tc.tile_pool(name="qp", bufs=1) as qp,        # [P, LB, H, 65] f16
            tc.tile_pool(name="expp", bufs=EXPP_BUFS) as expp,  # [P, 2, 512] f16
            tc.tile_pool(name="wp", bufs=4) as wp,
            tc.tile_pool(name="cst", bufs=1) as cst,
            tc.tile_pool(name="sm", bufs=16) as sm,       # small per-partition scalars
            tc.tile_pool(name="op", bufs=2) as outp,      # [P, 1024] out staging
            tc.tile_pool(name="pp2", bufs=PSUM_CFG[0], space="PSUM") as pp2,
            tc.tile_pool(name="pa", bufs=PSUM_CFG[1], space="PSUM") as pa,
            tc.tile_pool(name="pt", bufs=PSUM_CFG[2], space="PSUM") as pt,
        ):
            # ---- constants ----
            ident = cst.tile([P, P], f32, tag="ident")
            make_identity(nc, ident[:])
            tri = cst.tile([P, P], f16, tag="tri")  # keep j<=i
            make_upper_triangular(nc, tri[:], val=1.0, diag=True)
            eps_t = cst.tile([P, 1], f32, tag="eps")
            nc.vector.memset(eps_t[:], 1e-5)
            neg5_t = cst.tile([P, 1], f32, tag="neg5")
            nc.vector.memset(neg5_t[:], -5.0)
            if use_b1:
                b1_sb = cst.tile([P, NL, FC], f32, tag="b1")
                nc.sync.dma_start(b1_sb[:], b1.rearrange("l (c p) -> p l c", p=P))
            if use_bout:
                bout_b = cst.tile([P, V], f32, tag="bout")
                bout_ap = bout[:]
                nc.sync.dma_start(
                    bout_b[:],
                    bass.AP(tensor=bout_ap.tensor, offset=bout_ap.offset,
                            ap=[[0, P]] + bout_ap.ap),
                )

            def bcast_row(dram_row_ap, tag):
                t = cst.tile([P, F], f32, tag=tag)
                nc.sync.dma_start(
                    t[:],
                    bass.AP(tensor=dram_row_ap.tensor, offset=dram_row_ap.offset,
                            ap=[[0, P]] + dram_row_ap.ap),
                )
                return t

            import contextlib
            _loop = (tc.For_i(0, repeat, 1) if repeat > 1
                     else contextlib.nullcontext())
            with _loop:
                # ---- embedding gather ----
                tok_sb = cst.tile([P, LB], i32, tag="tok")
                nc.sync.dma_start(tok_sb[:], tokens.rearrange("(b p) -> p b", p=P))
                x_nat = natp.tile([P, LB, F], f32, tag="nat")
                if ABLATE == "embed":
                    nc.gpsimd.memset(x_nat[:], 0.02)
                else:
                    for b in range(LB):
                        nc.gpsimd.indirect_dma_start(
                            out=x_nat[:, b, :], out_offset=None,
                            in_=embed[:],
                            in_offset=bass.IndirectOffsetOnAxis(ap=tok_sb[:, b:b + 1], axis=0),
                        )

                def tcopy(i, dst, src):
                    """Merged psum->sbuf copy; engine per TCOPY."""
                    if TCOPY == "dve" or (TCOPY == "split" and i % 2 == 0):
                        nc.vector.tensor_copy(dst, src)
                    else:
                        nc.scalar.copy(dst, src)

                def transpose_blocks(src_nat, dst_T, b0, nb):
                    """Transpose l-blocks [b0, b0+nb) of natural [P, LB, F]
                    f32 into T layout [P, FC, L] (dtype cast per dst tile:
                    f32r or fp8). 4 transpose outputs share one psum bank and
                    drain with a single merged copy."""
                    if ABLATE == "transposes":
                        nc.gpsimd.memset(dst_T[:, :, b0 * P:(b0 + nb) * P], 0.1)
                        return
                    for b in range(b0, b0 + nb):
                        pt_ps = pt.tile([P, FC, P], f32, tag="pt")
                        for c in range(FC):
                            nc.tensor.transpose(
                                pt_ps[:, c, :],
                                src_nat[:, b, c * P:(c + 1) * P], ident[:])
                        tcopy(b, dst_T[:, :, b * P:(b + 1) * P], pt_ps[:])

                xT8 = t8p.tile([P, FC, L], f8, tag="t8")
                transpose_blocks(x_nat, xT8, 0, LB)

                # ---- layers ----
                for li in range(NL):
                    last = li == NL - 1
                    wq8_t = wp.tile([P, FC, HQ], f8, tag="w8", bufs=5 if MLP_FP8 else 3)
                    wk8_t = wp.tile([P, FC, HQ], f8, tag="w8", bufs=5 if MLP_FP8 else 3)
                    wv8_t = wp.tile([P, FC, HQ], f8, tag="w8", bufs=5 if MLP_FP8 else 3)
                    if ABLATE != "wdma":
                        nc.sync.dma_start(wq8_t[:], Wq8[li].rearrange("(c p) o -> p c o", p=P))
                        nc.sync.dma_start(wk8_t[:], Wk8[li].rearrange("(c p) o -> p c o", p=P))
                        nc.sync.dma_start(wv8_t[:], Wv8[li].rearrange("(c p) o -> p c o", p=P))
                    if MLP_FP8:
                        w1_t = wp.tile([P, FC, F], f8, tag="w8", bufs=5 if MLP_FP8 else 3)
                        w2_t = wp.tile([P, FC, F], f8, tag="w8", bufs=5 if MLP_FP8 else 3)
                        nc.sync.dma_start(
                            w1_t[:], W18[li].rearrange("(c p) o -> p c o", p=P))
                        nc.sync.dma_start(
                            w2_t[:], W28[li].rearrange("(c p) o -> p c o", p=P))
                    else:
                        w1_t = wp.tile([P, FC, F], f32r, tag="w", bufs=4)
                        w2_t = wp.tile([P, FC, F], f32r, tag="w", bufs=4)
                        if ABLATE != "wdma":
                            nc.sync.dma_start(
                                w1_t[:], _r(W1[li].rearrange("(c p) o -> p c o", p=P)))
                            nc.sync.dma_start(
                                w2_t[:], _r(W2[li].rearrange("(c p) o -> p c o", p=P)))

                    # kT, vT chunk oc: fp8 DoubleRow (256-deep K per pass),
                    # one [P,2,512] psum pair -> one 1024-wide f32r copy
                    kT = bigT.tile([P, FC, L], f32r, tag="bigT")
                    vT = bigT.tile([P, FC, L], f32r, tag="bigT")

                    def emit_kv(oc):
                        for wi, (w8_t, oT) in enumerate(((wk8_t, kT), (wv8_t, vT))):
                            ps = pp2.tile([P, 2, 512], f32, tag="pp2")
                            if DR_MODE == "dr":
                                # fc2 outer / lc inner: consecutive matmuls
                                # share the stationary -> one ldweights per
                                # weight block
                                for fc2 in range(0, FC, 2):
                                    for lc in range(2):
                                        nc.tensor.matmul(
                                            ps[:, lc, :],
                                            w8_t[:, fc2:fc2 + 2, oc * P:(oc + 1) * P],
                                            xT8[:, fc2:fc2 + 2, lc * 512:(lc + 1) * 512],
                                            start=(fc2 == 0), stop=(fc2 == FC - 2),
                                            perf_mode=DR)
                            else:
                                for fc in range(FC):
                                    for lc in range(2):
                                        nc.tensor.matmul(
                                            ps[:, lc, :],
                                            w8_t[:, fc, oc * P:(oc + 1) * P],
                                            xT8[:, fc, lc * 512:(lc + 1) * 512],
                                            start=(fc == 0), stop=(fc == FC - 1))
                            # chunk 0 drains on ACT: it gates the layer's
                            # first scores while DVE still holds the previous
                            # layer's transpose-copy backlog
                            eng = (nc.scalar.copy if oc == 0
                                   else nc.vector.tensor_copy)
                            eng(oT[:, oc, :],
                                ps[:].rearrange("p a b -> p (a b)"))

                    # q natural (fp16 for the attend matmul), [P(j), jc, head, 65]
                    # with a trailing ones column so attend also yields row-sums
                    q_sb = qp.tile([P, LB, H, 65], f16, tag="q")
                    nc.vector.memset(q_sb[:, :, :, 64:65], 1.0)

                    def emit_q2(bp):
                        ps = pp2.tile([P, 2, 512], f32, tag="pp2")
                        for i2 in range(2):
                            b = 2 * bp + i2
                            if DR_MODE == "dr":
                                for fc2 in range(0, FC, 2):
                                    nc.tensor.matmul(
                                        ps[:, i2, :],
                                        xT8[:, fc2:fc2 + 2, b * P:(b + 1) * P],
                                        wq8_t[:, fc2:fc2 + 2, :],
                                        start=(fc2 == 0), stop=(fc2 == FC - 2),
                                        perf_mode=DR)
                            else:
                                for fc in range(FC):
                                    nc.tensor.matmul(
                                        ps[:, i2, :],
                                        xT8[:, fc, b * P:(b + 1) * P],
                                        wq8_t[:, fc, :],
                                        start=(fc == 0), stop=(fc == FC - 1))
                        eng = nc.vector.tensor_copy if bp % 2 else nc.scalar.copy
                        eng(q_sb[:, 2 * bp:2 * bp + 2, :, 0:64],
                            ps[:].rearrange("p b (h d) -> p b h d", h=H))

                    x_new = natp.tile([P, LB, F], f32, tag="nat")
                    if MLP_FP8:
                        x_newT = t8p.tile([P, FC, L], f8, tag="t8")
                    else:
                        x_newT = bigT.tile([P, FC, L], f32r, tag="bigT")
                    exp_store: dict = {}

                    def emit_xnewT(p, half):
                        # transposes of x_new chunk p (head pair p's columns)
                        # for the 4 l-blocks finished by attend(p, half).
                        # Pair 3's copies gate MLP1 and run when the exps are
                        # done, so they drain on the then-idle ACT.
                        pt_ps = pt.tile([P, 4, P], f32, tag="pt")
                        for i, b in enumerate(range(4 * half, 4 * half + 4)):
                            nc.tensor.transpose(
                                pt_ps[:, i, :],
                                x_new[:, b, p * P:(p + 1) * P], ident[:])
                        eng = nc.scalar.copy if p == 3 else nc.vector.tensor_copy
                        eng(x_newT[:, p, 4 * half * P:(4 * half + 4) * P],
                            pt_ps[:])

                    def emit_scores(hpair, c):
                        heads = (2 * hpair, 2 * hpair + 1)
                        hc = hpair
                        tiles = {}
                        for jc in range(4 * c + 4):
                            d = jc - 4 * c
                            n0 = 0 if d < 0 else min(P * d, 256)
                            e0 = 0 if d < 0 else P * d
                            # both heads of the pair in one 2-bank psum tile
                            # so exp / affine_select run as single wide
                            # instructions (halves the per-instr ACT access
                            # latency spend)
                            ps = pp2.tile([P, 2, 512], f32, tag="pp2")
                            if ABLATE != "scores":
                                for hi, h in enumerate(heads):
                                    hb = 64 * (h % 2)
                                    nc.tensor.matmul(
                                        ps[:, hi, n0:512],
                                        vT[hb:hb + KD, hc, jc * P:(jc + 1) * P],
                                        kT[hb:hb + KD, hc, c * 512 + n0:(c + 1) * 512],
                                        start=True, stop=True)
                            et = expp.tile([P, 2, 512], f16, tag="exp")
                            # bias=-5: softmax is shift-invariant (both the
                            # attend numerator and the ones-column row-sum
                            # scale by e^-5), keeps exp within fp16 range
                            if ABLATE != "scores":
                                nc.scalar.activation(
                                    et[:, :, e0:512], ps[:, :, e0:512], AF.Exp,
                                    bias=neg5_t[:])
                                if d >= 0:
                                    # zero att where j > i, both heads at once
                                    if TRIMASK == "dve":
                                        nc.vector.tensor_tensor(
                                            et[:, :, e0:e0 + P],
                                            et[:, :, e0:e0 + P],
                                            tri[:, None, :].to_broadcast(
                                                (P, 2, P)),
                                            OP.mult)
                                    else:
                                        nc.gpsimd.affine_select(
                                            out=et[:, :, e0:e0 + P],
                                            in_=et[:, :, e0:e0 + P],
                                            compare_op=OP.is_ge,
                                            fill=0.0, base=0,
                                            pattern=[[0, 2], [1, P]],
                                            channel_multiplier=-1)
                            else:
                                nc.gpsimd.memset(et[:, :, e0:512], 0.5)
                            tiles[jc] = et
                        exp_store[(hpair, c)] = tiles

                    def emit_attend(hpair, c):
                        heads = (2 * hpair, 2 * hpair + 1)
                        tiles = exp_store.pop((hpair, c))
                        # two b-slots share one psum bank (2 x 130 f32);
                        # normalization is batched per b-pair: one strided
                        # reciprocal + one broadcast multiply for both slots
                        pa_t = pa.tile([P, 2, 130], f32, tag="pa")
                        for b0 in range(4 * c, 4 * c + 4, 2):
                            if ABLATE == "attend":
                                for h in heads:
                                    nc.gpsimd.memset(
                                        x_new[:, b0:b0 + 2,
                                              h * 64:(h + 1) * 64], 0.1)
                                continue
                            for s, b in enumerate((b0, b0 + 1)):
                                lc0 = (b - 4 * c) * P
                                # both heads of the pair accumulate into one
                                # psum bank: head h' at cols [65*h', 65*h'+65)
                                for hi, h in enumerate(heads):
                                    for jc in range(b + 1):
                                        nc.tensor.matmul(
                                            pa_t[:, s, 65 * hi:65 * hi + 65],
                                            tiles[jc][:, hi, lc0:lc0 + P],
                                            q_sb[:, jc, h, :],
                                            start=(jc == 0), stop=(jc == b))
                            pa4 = pa_t[:].rearrange("p s (h x) -> p s h x", h=2)
                            rc = sm.tile([P, 2, 2], f32, tag="rc")
                            nc.vector.reciprocal(rc[:], pa4[:, :, :, 64])
                            # x_new[:, b0:b0+2, pair] = att_u @ q * recip
                            # (recip broadcast 64-wide per head, 0-stride)
                            xdst = x_new[:, b0:b0 + 2,
                                         hpair * P:(hpair + 1) * P].rearrange(
                                "p b (h x) -> p b h x", h=2)
                            nc.vector.tensor_tensor(
                                xdst, pa4[:, :, :, 0:64],
                                rc[:, :, :, None].to_broadcast((P, 2, 2, 64)),
                                OP.mult)

                    # schedule: kv chunk p -> attends of pair p-1 (+ their
                    # transposes) -> scores of pair p. Scores sit last in each
                    # iteration because the pp2 ring throttles them to ACT's
                    # exp pace — everything PE-independent is emitted first.
                    for p in range(H // 2):
                        emit_kv(p)
                        if p == 0:
                            emit_scores(0, 0)
                            emit_scores(0, 1)
                            emit_q2(0)
                            emit_q2(1)
                        else:
                            if p == 1:
                                emit_q2(2)
                                emit_q2(3)
                            emit_attend(p - 1, 0)
                            emit_xnewT(p - 1, 0)
                            emit_attend(p - 1, 1)
                            emit_xnewT(p - 1, 1)
                            emit_scores(p, 0)
                            emit_scores(p, 1)
                    emit_attend(3, 0)
                    emit_xnewT(3, 0)
                    emit_attend(3, 1)
                    emit_xnewT(3, 1)

                    # MLP1: h1T = relu(W1^T x_newT + b1), one [P,2,512] psum
                    # pair + one wide ACT relu per out chunk; fc outer / lc
                    # inner so consecutive matmuls share the stationary
                    if MLP_FP8:
                        h1T = t8p.tile([P, FC, L], f8, tag="t8")
                    else:
                        h1T = bigT.tile([P, FC, L], f32r, tag="bigT")
                    for oc in range(FC):
                        ps = pp2.tile([P, 2, 512], f32, tag="pp2")
                        if MLP_FP8:
                            for fc2 in range(0, FC, 2):
                                for lc in range(2):
                                    nc.tensor.matmul(
                                        ps[:, lc, :],
                                        w1_t[:, fc2:fc2 + 2, oc * P:(oc + 1) * P],
                                        x_newT[:, fc2:fc2 + 2,
                                               lc * 512:(lc + 1) * 512],
                                        start=(fc2 == 0), stop=(fc2 == FC - 2),
                                        perf_mode=DR)
                        else:
                            for fc in range(FC):
                                for lc in range(2):
                                    nc.tensor.matmul(
                                        ps[:, lc, :],
                                        w1_t[:, fc, oc * P:(oc + 1) * P],
                                        x_newT[:, fc, lc * 512:(lc + 1) * 512],
                                        start=(fc == 0), stop=(fc == FC - 1))
                        bias = b1_sb[:, li, oc:oc + 1] if use_b1 else 0.0
                        nc.scalar.activation(
                            h1T[:, oc, :], ps[:].rearrange("p a b -> p (a b)"),
                            AF.Relu, bias=bias)

                    # MLP2 + residual + LN -> y, l-blocks in pairs, with the
                    # y transposes (and, last layer, the unembed) interleaved
                    # per pair so the layer-boundary chain stays pipelined
                    if use_b2:
                        b2_b = bcast_row(b2[li], f"b2_{li}")
                    if use_gamma:
                        gamma_b = bcast_row(gamma[li], f"g_{li}")
                    if use_beta:
                        beta_b = bcast_row(beta[li], f"be_{li}")
                    if last:
                        wo = []
                        for vc in range(2):
                            wt = wp.tile([P, FC, 512], f32r, tag="w", bufs=4)
                            nc.sync.dma_start(
                                wt[:],
                                _r(Wout[:, vc * 512:(vc + 1) * 512]
                                   .rearrange("(c p) o -> p c o", p=P)))
                            wo.append(wt)
                        xT_next = bigT.tile([P, FC, L], f32r, tag="bigT")
                    else:
                        xT_next = t8p.tile([P, FC, L], f8, tag="t8")
                    y = natp.tile([P, LB, F], f32, tag="nat")
                    mv8 = sm.tile([P, LB, 2], f32, tag="mv8")
                    rstd8 = sm.tile([P, LB], f32, tag="rs8")

                    def emit_unembed(b):
                        # fc outer / vc inner: consecutive matmuls share the
                        # stationary xT block -> one ldweights per fc
                        ps = pp2.tile([P, 2, 512], f32, tag="pp2")
                        for fc in range(FC):
                            for vc in range(2):
                                nc.tensor.matmul(
                                    ps[:, vc, :],
                                    xT_next[:, fc, b * P:(b + 1) * P],
                                    wo[vc][:, fc, :],
                                    start=(fc == 0), stop=(fc == FC - 1))
                        if ABLATE == "outdma":
                            return
                        ot = outp.tile([P, V], f32, tag="o")
                        psf = ps[:].rearrange("p a b -> p (a b)")
                        if use_bout:
                            nc.vector.tensor_add(ot[:], psf, bout_b[:])
                        else:
                            eng = (nc.vector.tensor_copy if b % 2
                                   else nc.scalar.copy)
                            eng(ot[:], psf)
                        nc.sync.dma_start(out[b * P:(b + 1) * P, :], ot[:])

                    for bp in range(LB // 2):
                        ps = pp2.tile([P, 2, 512], f32, tag="pp2")
                        for i2 in range(2):
                            b = 2 * bp + i2
                            if MLP_FP8:
                                for fc2 in range(0, FC, 2):
                                    nc.tensor.matmul(
                                        ps[:, i2, :],
                                        h1T[:, fc2:fc2 + 2, b * P:(b + 1) * P],
                                        w2_t[:, fc2:fc2 + 2, :],
                                        start=(fc2 == 0), stop=(fc2 == FC - 2),
                                        perf_mode=DR)
                            else:
                                for fc in range(FC):
                                    nc.tensor.matmul(
                                        ps[:, i2, :],
                                        h1T[:, fc, b * P:(b + 1) * P],
                                        w2_t[:, fc, :],
                                        start=(fc == 0), stop=(fc == FC - 1))
                        t2 = y[:, 2 * bp:2 * bp + 2, :]
                        nc.vector.tensor_add(t2, ps[:], x_nat[:, 2 * bp:2 * bp + 2, :])
                        if use_b2:
                            nc.vector.tensor_add(
                                t2, t2,
                                b2_b[:, None, :].to_broadcast((P, 2, F)))
                        for i2 in range(2):
                            b = 2 * bp + i2
                            if ABLATE == "ln":
                                continue
                            st = sm.tile([P, 6], f32, tag="st")
                            nc.vector.bn_stats(st[:], y[:, b, :])
                            nc.vector.bn_aggr(mv8[:, b, :], st[:])
                    # batched rstd = exp(-0.5*ln(var+eps)), 4 blocks per
                    # [P,4] ACT pair instead of 16 tiny per-block ops — the
                    # hw charges ~us-scale latency per cross-engine cascade,
                    # so two cascades per layer beat a pipelined-looking
                    # per-block chain by ~50us/layer (measured via ablation).
                    # Split in halves so blocks 0-3 (which gate the next
                    # layer's first K/V chunk) normalize without waiting for
                    # blocks 4-7's stats.
                    for half in range(2):
                        h0 = 4 * half
                        if ABLATE != "ln":
                            nc.scalar.activation(
                                rstd8[:, h0:h0 + 4], mv8[:, h0:h0 + 4, 1],
                                AF.Ln, bias=eps_t[:])
                            nc.scalar.activation(
                                rstd8[:, h0:h0 + 4], rstd8[:, h0:h0 + 4],
                                AF.Exp, scale=-0.5)
                            for b in range(h0, h0 + 4):
                                _ln_apply(nc, y, b, mv8, rstd8, use_gamma,
                                          use_beta,
                                          gamma_b if use_gamma else None,
                                          beta_b if use_beta else None)
                        transpose_blocks(y, xT_next, h0, 4)
                        if last:
                            for b in range(h0, h0 + 4):
                                emit_unembed(b)

                    x_nat = y
                    xT8 = xT_next
    nc.compile()
    return nc


def _get_nc(flags, repeat=1):
    key = (flags, repeat, ABLATE, LN_BATCH, PSUM_CFG, TCOPY, EXPP_BUFS,
           DR_MODE, MLP_FP8, TRIMASK)
    if key not in _NC_CACHE:
        _NC_CACHE[key] = _build(flags, repeat)
    return _NC_CACHE[key]


def make_runner(flags, in_maps, repeat=1):
    """Build a reusable jitted SPMD runner with device-resident inputs.

    Returns (run, split_outputs) where run() executes the kernel once on all
    8 cores and blocks; used by test.py for timing without per-call host->device
    input transfer.
    """
    import jax
    from jax.sharding import Mesh, PartitionSpec, NamedSharding
    from concourse import bass2jax, mybir as _mybir

    bass2jax.install_neuronx_cc_hook()
    nc = _get_nc(flags, repeat)
    partition_name = (nc.partition_id_tensor.name if nc.partition_id_tensor
                      else None)
    in_names, out_names, out_avals, zero_outs = [], [], [], []
    for alloc in nc.m.functions[0].allocations:
        if not isinstance(alloc, _mybir.MemoryLocationSet):
            continue
        name = alloc.memorylocations[0].name
        if alloc.kind == "ExternalInput":
            if name != partition_name:
                in_names.append(name)
        elif alloc.kind == "ExternalOutput":
            shape = tuple(alloc.tensor_shape)
            dtype = _mybir.dt.np(alloc.dtype)
            out_names.append(name)
            out_avals.append(jax.core.ShapedArray(shape, dtype))
            zero_outs.append(np.zeros(shape, dtype))
    n_params = len(in_names)
    n_outs = len(out_avals)
    all_names = in_names + out_names + ([partition_name] if partition_name else [])

    def _body(*args):
        operands = list(args)
        if partition_name is not None:
            operands.append(bass2jax.partition_id_tensor())
        outs = bass2jax._bass_exec_p.bind(
            *operands,
            out_avals=tuple(out_avals),
            in_names=tuple(all_names),
            out_names=tuple(out_names),
            lowering_input_output_aliases=(),
            sim_require_finite=True,
            sim_require_nnan=True,
            nc=nc,
        )
        return tuple(outs)

    from jax.experimental.shard_map import shard_map
    devices = jax.devices()[:NCORES]
    mesh = Mesh(np.asarray(devices), ("core",))
    in_specs = (PartitionSpec("core"),) * (n_params + n_outs)
    out_specs = (PartitionSpec("core"),) * n_outs
    sharded = jax.jit(
        shard_map(_body, mesh=mesh, in_specs=in_specs, out_specs=out_specs,
                  check_rep=False),
        keep_unused=True,
    )
    concat_in = [
        np.concatenate([np.asarray(in_maps[c][nm])[None] for c in range(NCORES)],
                       axis=0).reshape(NCORES * np.asarray(in_maps[0][nm]).shape[0],
                                       *np.asarray(in_maps[0][nm]).shape[1:])
        for nm in in_names
    ]
    sh = NamedSharding(mesh, PartitionSpec("core"))
    dev_in = [jax.device_put(x, sh) for x in concat_in]
    dev_zeros = [
        jax.device_put(np.zeros((NCORES * z.shape[0], *z.shape[1:]), z.dtype), sh)
        for z in zero_outs
    ]

    def run():
        outs = sharded(*dev_in, *dev_zeros)
        jax.block_until_ready(outs)
        return outs

    def split(outs):
        return [
            {nm: np.asarray(outs[i]).reshape(NCORES, *out_avals[i].shape)[c]
             for i, nm in enumerate(out_names)}
            for c in range(NCORES)
        ]

    return run, split


def prep_args(inputs):
    """Host-side arg prep shared by kernel() and test.py: fp32 copies of the
    fp32 params, fp8e4m3 casts of Wq/Wk/Wv (consumed by the DoubleRow
    projection matmuls)."""
    import ml_dtypes
    args = {k: np.ascontiguousarray(np.asarray(v), dtype=np.float32)
            for k, v in inputs.items() if k not in ("tokens", "Wq", "Wk", "Wv")}
    for k in ("Wq", "Wk", "Wv"):
        args[k + "8"] = np.ascontiguousarray(
            np.asarray(inputs[k], dtype=np.float32).astype(ml_dtypes.float8_e4m3))
    if MLP_FP8:
        for k in ("W1", "W2"):
            args[k + "8"] = np.ascontiguousarray(
                args[k].astype(ml_dtypes.float8_e4m3))
    return args


def kernel(**inputs) -> np.ndarray:
    tokens = np.asarray(inputs["tokens"])
    args = prep_args(inputs)
    flags = (
        bool(np.any(args["b1"])),
        bool(np.any(args["b2"])),
        bool(np.any(args["gamma"] != 1.0)),
        bool(np.any(args["beta"])),
        bool(np.any(args["bout"])),
    )
    nc = _get_nc(flags)
    tok32 = np.ascontiguousarray(tokens.astype(np.int32))
    in_maps = [dict(args, tokens=tok32[c]) for c in range(NCORES)]
    res = run_bass_kernel_spmd(nc, in_maps, list(range(NCORES)))
    return np.stack([res.results[c]["out"] for c in range(NCORES)], axis=0)


if __name__ == "__main__":
    rng = np.random.default_rng(0)
    toy = {
        "tokens": rng.integers(0, V, size=(N, L)),
        "embed": rng.standard_normal((V, F)).astype(np.float32) * 0.02,
        "Wq": rng.standard_normal((NL, F, HQ)).astype(np.float32) * 0.02,
        "Wk": rng.standard_normal((NL, F, H * KD)).astype(np.float32) * 0.02,
        "Wv": rng.standard_normal((NL, F, H * KD)).astype(np.float32) * 0.02,
        "W1": rng.standard_normal((NL, HQ, F)).astype(np.float32) * 0.02,
        "b1": np.zeros((NL, F), np.float32),
        "W2": rng.standard_normal((NL, F, F)).astype(np.float32) * 0.02,
        "b2": np.zeros((NL, F), np.float32),
        "gamma": np.ones((NL, F), np.float32),
        "beta": np.zeros((NL, F), np.float32),
        "Wout": rng.standard_normal((F, V)).astype(np.float32) * 0.02,
        "bout": np.zeros((V,), np.float32),
    }
    o = kernel(**toy)
    print("out:", o.shape, o.dtype, float(np.abs(o).max()))
